# revision 76
# baseline (speedup 1.0000x reference)
"""DMSTGCN forward on 8 Trainium2 NeuronCores (Bass/Tile).

Sharding: data-parallel over batch B=16 -> 2 batches per core; parameters
replicated. The dynamic adjacency (1024x1024 per batch) is built on device
(clamp evictions split DVE/Act, the Act half as a sigmoid step) and held in
SBUF as fp8e4 in a DoubleRow K-pair layout (128, 2, N); all three hop matmul
groups run as fp8 DoubleRow (0.5 cycles/row, 256-deep contraction per
instruction). Hop1 produces node-major (V) output that hop2 consumes directly
as a DoubleRow stationary, emitting channel-major (T) output - so no PE
back-transposes are needed. The gconv consumes an fp8 (h1*S1, xa) pair via
one DoubleRow matmul (folding the per-layer xa diagonal) plus bf16 matmuls
for the xn and h2 terms; h2 stays bf16 because its fp8 quantization breaks
the error budget (h2max*W2max > 240^2). The attention fc2 also runs as an
fp8 DoubleRow pair over (r1/256, zero-weighted filler). Start convs are
computed on the host and shipped; phase0-critical inputs are packed into
single DMAs (each DMA carries ~2.2us fixed latency). end1 skip
contributions accumulate into a batch-stacked (128, N) SBUF tensor: the
end1 matmuls write both batches into partition halves of one PSUM tile so
a single DVE op per column-half accumulates them. Trunk math stays float32r (fp8/bf16 trunk compounds ~3%/layer through
the sigmoid boundary and is not safe). Eviction engines are hand-balanced
between DVE and Act; the tail runs all o1 on Act because DVE is backlogged
with the layer-7 end1 accumulates.
"""
import numpy as np
import ml_dtypes

import concourse.bacc as bacc
import concourse.mybir as mybir
from concourse.tile import TileContext
from concourse.bass_utils import run_bass_kernel_spmd

F32 = mybir.dt.float32
F32R = mybir.dt.float32r
BF16 = mybir.dt.bfloat16
F8 = mybir.dt.float8e4
F8E5 = mybir.dt.float8e5
AF = mybir.ActivationFunctionType
ALU = mybir.AluOpType
DR = mybir.MatmulPerfMode.DoubleRow

B, N, T, RF = 16, 1024, 12, 13
RC, SC, DIMS, L = 16, 8, 32, 8
BN_EPS = 1e-5
NCORES = 8
BPC = B // NCORES          # batches per core
CL = RC * RF               # 208 rows in T-layout
SKR = SC * RF              # 104 skip rows
CH = ((0, 128), (128, 80))  # l-major T-layout row chunks
CLS = (slice(0, 128), slice(128, 208))  # CL slices per chunk
NV_COLS = L + L * 2 * 3 + 3
S1 = 1.0 / 32.0            # h1 storage scale (fp8)
SR = 1.0 / 256.0           # r1 storage scale (fp8)
S2 = 1.0 / 64.0            # extra h2 eviction scale (net h2/2048)
GM = 1.0                   # gconv PSUM scale (h2 stays bf16)

_CACHED = None


def _build_nc():
    nc = bacc.Bacc("TRN2", target_bir_lowering=False)

    d = {}
    def din(name, shape, dt=F32R):
        d[name] = nc.dram_tensor(name, list(shape), dt, kind="ExternalInput")

    din("x0c0", (BPC, 128, N))
    din("x0c1", (BPC, 80, N))
    din("xa8c0", (BPC, 128, N), F8)
    din("xa8c1", (BPC, 80, N), F8)
    din("ph0", (DIMS, N + DIMS + BPC * DIMS))
    din("wfp", (128, 336))
    din("idenb", (128, 128), BF16)
    din("gcw0_0", (128, L * 128), BF16)
    din("gcw0_1", (80, L * 80), BF16)
    din("wg12_0", (128, L * 128), BF16)
    din("wg12_1", (80, L * 80), BF16)
    din("wh1a_0", (128, L * 2 * 128), F8)
    din("wh1a_1", (80, L * 2 * 80), F8)
    din("wfc2p_0", (128, 2 * 128), F8)
    din("wfc2p_1", (80, 2 * 80), F8)
    din("wskip_0", (128, L * 64), BF16)
    din("wskip_1", (80, L * 40), BF16)
    din("we1", (SKR, L * 64), BF16)
    din("we2t", (128, 12))
    din("vecs", (128, NV_COLS), F32)
    outp = nc.dram_tensor("outp", [BPC, 12, N], F32, kind="ExternalOutput")

    with TileContext(nc) as tc, \
         tc.tile_pool(name="wp", bufs=1) as wp, \
         tc.tile_pool(name="ap", bufs=1) as ap, \
         tc.tile_pool(name="pp", bufs=1, space="PSUM") as pp:

        def wtile(name, src_ap, shape, dt=F32R, eng=None):
            t = wp.tile(shape, dt, tag=name, name=name)
            (eng or nc.sync).dma_start(out=t[:], in_=src_ap)
            return t

        # phase0-critical loads packed into ONE DMA on the SP queue (each
        # DMA carries ~2.2us fixed latency); bulk weights on gpsimd.
        ph0 = wtile("ph0", d["ph0"][:], (DIMS, N + DIMS + BPC * DIMS),
                    eng=nc.sync)
        p2T = ph0[:, 0:N]
        p3sT = ph0[:, N:N + DIMS]
        adps = [ph0[:, N + DIMS + b * DIMS:N + DIMS + (b + 1) * DIMS]
                for b in range(BPC)]

        # trunk/attention tiles; x0 DMA'd straight into the first XT
        # buffers, with the att weights interleaved so att(0, b0) can start
        # before batch 1's x0 lands.
        xts = [[None, None] for _ in range(BPC)]
        for b in range(BPC):
            for c in range(2):
                rows = CH[c][1]
                xts[b][c] = ap.tile((rows, N), F32R, tag=f"XT{b}_{c}",
                                    bufs=2, name=f"XT{b}_{c}_init")
        nc.sync.dma_start(out=xts[0][0][:], in_=d["x0c0"][0])
        wfp = wtile("wfp", d["wfp"][:], (128, 336))
        wfc1 = [wfp[:, 0:128], wfp[:80, 128:208]]
        idenh = wfp[:, 208:336]
        nc.sync.dma_start(out=xts[0][1][:], in_=d["x0c1"][0])
        nc.sync.dma_start(out=xts[1][0][:], in_=d["x0c0"][1])
        nc.sync.dma_start(out=xts[1][1][:], in_=d["x0c1"][1])
        idenb = wtile("idenb", d["idenb"][:], (128, 128), BF16)
        vecs = wtile("vecs", d["vecs"][:], (128, NV_COLS), F32)

        php = [[ap.tile((CH[c][1], 2, N), F8, tag=f"PH{b}_{c}",
                        name=f"PH{b}_{c}") for c in range(2)]
               for b in range(BPC)]
        r1p = [[ap.tile((CH[c][1], 2, N), F8, tag=f"R1{b}_{c}",
                        name=f"R1{b}_{c}") for c in range(2)]
               for b in range(BPC)]

        h2t = [[ap.tile((CH[c][1], N), BF16, tag=f"H2{b}_{c}",
                        name=f"H2{b}_{c}") for c in range(2)]
               for b in range(BPC)]
        wfc2p = [wtile(f"wfc2p_{c}", d[f"wfc2p_{c}"][:],
                       (CH[c][1], 2, CH[c][1]), F8, eng=nc.gpsimd)
                 for c in range(2)]
        for b in range(BPC):
            for c in range(2):
                nc.gpsimd.dma_start(out=php[b][c][:, 1, :],
                                    in_=d[f"xa8c{c}"][b])
        gcw0 = [wtile(f"gcw0_{c}", d[f"gcw0_{c}"][:],
                      (CH[c][1], L, CH[c][1]), BF16, eng=nc.gpsimd)
                for c in range(2)]
        wg12 = [wtile(f"wg12_{c}", d[f"wg12_{c}"][:],
                      (CH[c][1], L, CH[c][1]), BF16, eng=nc.gpsimd)
                for c in range(2)]
        wh1a = [wtile(f"wh1a_{c}", d[f"wh1a_{c}"][:],
                      (CH[c][1], L, 2, CH[c][1]), F8, eng=nc.gpsimd)
                for c in range(2)]
        wskip = [wtile(f"wskip_{c}", d[f"wskip_{c}"][:],
                       (CH[c][1], L, (64, 40)[c]), BF16, eng=nc.gpsimd)
                 for c in range(2)]
        we1 = wtile("we1", d["we1"][:], (SKR, L, 64), BF16, eng=nc.gpsimd)
        we2t = wtile("we2t", d["we2t"][:], (128, 12), eng=nc.gpsimd)

        vc = {}
        ci = 0
        for i in range(L):
            vc[f"skb{i}"] = ci; ci += 1
        for i in range(L):
            for c in range(2):
                for nm in ("bns", "bnb", "bnsg"):
                    vc[f"{nm}{i}_{c}"] = ci; ci += 1
        vc["e1b"] = ci; ci += 1
        vc["e2b"] = ci; ci += 1
        vc["neg20"] = ci; ci += 1
        assert ci == NV_COLS

        def vcol(nm, rows=128):
            return vecs[:rows, vc[nm]:vc[nm] + 1]

        NS = (slice(0, 512), slice(512, 1024))
        BS = range(BPC)

        st = [dict() for _ in range(BPC)]
        # persistent fp8 adjacency in DoubleRow pair layout
        ATd = [[ap.tile((128, 2, N), F8, tag=f"ATd{b}_{k}", name=f"ATd{b}_{k}")
                for k in range(4)] for b in BS]
        ends = ap.tile((128, N), F32, tag="END", name="END")

        # PE p-state warmup: tiny matmuls as early as possible so the
        # 3us ramp window elapses before the heavy phase0 matmuls.
        def warmup():
            wps = pp.tile((DIMS, 8), F32, tag="pwork", bufs=4, name="warm")
            for r in range(6):
                nc.tensor.matmul(wps[:], adps[0][:], p2T[:, 0:8],
                                 start=(r == 0), stop=(r == 5))

        # ---------------- adjacency ----------------
        def phase0_pair():
            Lst = [ap.tile((64, N), F32R, tag=f"Lst{b}", name=f"Lst{b}")
                   for b in BS]
            Rst = [ap.tile((64, N), F32R, tag=f"Rst{b}", name=f"Rst{b}")
                   for b in BS]
            for nsi, ns in enumerate(NS):
                pss = []
                for b in BS:
                    ps = pp.tile((DIMS, 512), F32, tag="pwork", bufs=4,
                                 name=f"srcTps{b}_{nsi}")
                    nc.tensor.matmul(ps[:], adps[b], p2T[:, ns],
                                     start=True, stop=True)
                    pss.append(ps)
                for b in BS:
                    nc.scalar.activation(Rst[b][0:32, ns], pss[b][:], AF.Copy)
                    nc.vector.tensor_scalar(Lst[b][32:64, ns], pss[b][:],
                                            -1.0, None, ALU.mult)
            for nsi, ns in enumerate(NS):
                pss = []
                for b in BS:
                    ps = pp.tile((DIMS, 512), F32, tag="pwork", bufs=4,
                                 name=f"ups{b}_{nsi}")
                    nc.tensor.matmul(ps[:], p3sT, Rst[b][0:32, ns],
                                     start=True, stop=True)
                    pss.append(ps)
                for b in BS:
                    nc.scalar.activation(Lst[b][0:32, ns], pss[b][:], AF.Copy)
                    nc.vector.tensor_copy(Rst[b][32:64, ns], pss[b][:])
            st[0]["LR"] = (Lst, Rst)

        def phase0_D(fillers=()):
            fillers = list(fillers)
            Lst, Rst = st[0]["LR"]
            for v in range(8):
                cs = slice(v * 128, (v + 1) * 128)
                for nsi, ns in enumerate(NS):
                    dpss = []
                    for b in BS:
                        dps = pp.tile((128, 512), F32, tag="pwork", bufs=4,
                                      name=f"dps{b}_{v}_{nsi}")
                        nc.tensor.matmul(dps[:], Lst[b][:, cs], Rst[b][:, ns],
                                         start=True, stop=True)
                        dpss.append(dps)
                    # A = clamp(D, 0, 1): |D| ~ 5e4 >> 1 and off-diagonal
                    # |D| >= 0.03, so clamp == step(D>0) except on a ~2e-5
                    # sliver; diag D == 0 maps to 0 via the -20 bias. Act
                    # takes half the evictions as sigmoid(1e6 D - 20).
                    for b in BS:
                        if (b + v + nsi) % 2 == 0:
                            nc.vector.tensor_scalar(
                                ATd[b][v // 2][:, v % 2, ns], dpss[b][:],
                                0.0, 1.0, ALU.max, ALU.min)
                        else:
                            nc.scalar.activation(
                                ATd[b][v // 2][:, v % 2, ns], dpss[b][:],
                                AF.Sigmoid, bias=vcol("neg20"),
                                scale=1e6)
                if v % 2 == 1 and fillers:
                    fillers.pop(0)()
            while fillers:
                fillers.pop(0)()

        # ---------------- layer stages ----------------
        def att_c(i, c):
            """fc1 -> relu -> fc2 + x/2 -> sigmoid(2.) for one chunk."""
            rows = CH[c][1]
            xn = {}
            for b in BS:
                if c == 0:
                    st[b]["xn_next"] = [None, None]
                xn[b] = ap.tile((rows, N), BF16, tag=f"XN{b}_{c}",
                                bufs=2, name=f"XN{b}_{i}_{c}")
                st[b]["xn_next"][c] = xn[b]
            m1s, apss = {}, {}
            for nsi in range(2):
                ns = NS[nsi]
                for b in BS:
                    xt = st_xt(b)
                    m1 = pp.tile((rows, 512), F32, tag="pwork", bufs=4,
                                 name=f"m1_{b}_{i}_{c}_{nsi}")
                    nc.tensor.matmul(m1[:], wfc1[c], xt[c][:, ns],
                                     start=True, stop=True)
                    m1s[b, nsi] = m1
                for b in BS:
                    if b % 2 == 0:
                        nc.scalar.activation(r1p[b][c][:, 0, ns],
                                             m1s[b, nsi][:],
                                             AF.Relu, scale=SR)
                    else:
                        nc.vector.tensor_scalar(r1p[b][c][:, 0, ns],
                                                m1s[b, nsi][:],
                                                SR, 0.0,
                                                ALU.mult, ALU.max)
            for nsi in range(2):
                ns = NS[nsi]
                for b in BS:
                    xt = st_xt(b)
                    a_ps = pp.tile((rows, 512), F32, tag="pwork", bufs=4,
                                   name=f"aps{b}_{i}_{c}_{nsi}")
                    nc.tensor.matmul(a_ps[:], wfc2p[c][:, :, :],
                                     r1p[b][c][:, :, ns], perf_mode=DR,
                                     start=True, stop=False)
                    nc.tensor.matmul(a_ps[:], idenh[:rows, 0:rows],
                                     xt[c][:, ns], start=False, stop=True)
                    apss[b, nsi] = a_ps
                for b in BS:
                    nc.scalar.activation(xn[b][:, ns], apss[b, nsi][:],
                                         AF.Sigmoid, scale=2.0)

        def att(i):
            att_c(i, 0)
            att_c(i, 1)

        def st_xt(b):
            if "xt" not in st[b]:
                st[b]["xt"] = [xts[b][0], xts[b][1]]
            return st[b]["xt"]

        def tpx(i, b):
            """xn -> V-layout fp8 DoubleRow pairs xvd[kp] (128,2,CL)."""
            st[b]["xn"] = st[b]["xn_next"]
            xn = st[b]["xn"]
            xvd = [None] * 4
            for kp in range(4):
                tp = pp.tile((128, 2, CL), BF16, tag="ptr", bufs=2,
                             name=f"tpx{b}_{i}_{kp}")
                for s in range(2):
                    v = 2 * kp + s
                    cs = slice(v * 128, (v + 1) * 128)
                    for c in range(2):
                        o, rows = CH[c]
                        nc.tensor.transpose(tp[:, s, o:o + rows],
                                            xn[c][:, cs], idenb[:rows, :rows])
                xvd[kp] = ap.tile((128, 2, CL), F8, tag=f"XV{b}_{kp}",
                                  bufs=2, name=f"XV{b}_{i}_{kp}")
                if kp % 2 == 0:
                    nc.vector.tensor_copy(xvd[kp][:], tp[:])
                else:
                    nc.scalar.activation(xvd[kp][:], tp[:], AF.Copy)
            st[b]["xvd"] = xvd

        def hop1v_pb(i, p, b):
            """h1 V-pair for one w-pair p, one batch."""
            if p == 0:
                st[b]["h1d"] = [None] * 4
            xvd = st[b]["xvd"]
            h_ps = pp.tile((128, 2, CL), F32, tag="ptr", bufs=2,
                           name=f"hp{b}_{i}_{p}")
            for s in range(2):
                w = 2 * p + s
                ws = slice(w * 128, (w + 1) * 128)
                dst = h_ps[:, s, :]
                for kp in range(4):
                    nc.tensor.matmul(dst, ATd[b][kp][:, :, ws],
                                     xvd[kp][:], perf_mode=DR,
                                     start=(kp == 0), stop=(kp == 3))
            t = ap.tile((128, 2, CL), F8, tag=f"H1{b}_{p}",
                        bufs=2, name=f"H1{b}_{i}_{p}")
            if p < 3:
                nc.scalar.activation(t[:], h_ps[:], AF.Identity,
                                     scale=S1)
            else:
                nc.vector.tensor_scalar(t[:], h_ps[:], S1, None,
                                        ALU.mult)
            st[b]["h1d"][p] = t

        def hop1t_unit(i, c, nsi):
            rows, ns = CH[c][1], NS[nsi]
            g1s = {}
            for b in BS:
                xvd = st[b]["xvd"]
                g1 = pp.tile((rows, 512), F32, tag="pwork", bufs=4,
                             name=f"g1_{b}_{i}_{c}_{nsi}")
                for kp in range(4):
                    nc.tensor.matmul(g1[:], xvd[kp][:, :, CLS[c]],
                                     ATd[b][kp][:, :, ns], perf_mode=DR,
                                     start=(kp == 0), stop=(kp == 3))
                g1s[b] = g1
            for b in BS:
                dst = php[b][c][:, 0, ns]
                if (b + nsi) % 2 == 0 or c == 0:
                    nc.scalar.activation(dst, g1s[b][:], AF.Identity,
                                         scale=S1)
                else:
                    nc.vector.tensor_scalar(dst, g1s[b][:], S1, None,
                                            ALU.mult)

        def hop2t_unit(i, c, nsi):
            rows, ns = CH[c][1], NS[nsi]
            g2s = {}
            for b in BS:
                h1d = st[b]["h1d"]
                g2 = pp.tile((rows, 512), F32, tag="pwork", bufs=4,
                             name=f"g2_{b}_{i}_{c}_{nsi}")
                for kp in range(4):
                    nc.tensor.matmul(g2[:], h1d[kp][:, :, CLS[c]],
                                     ATd[b][kp][:, :, ns], perf_mode=DR,
                                     start=(kp == 0), stop=(kp == 3))
                g2s[b] = g2
            for b in BS:
                dst = h2t[b][c][:, ns]
                if (b + c + nsi) % 2 == 1:
                    nc.scalar.activation(dst, g2s[b][:], AF.Copy)
                else:
                    nc.vector.tensor_copy(dst, g2s[b][:])

        def skp_part(i):
            """skip conv -> relu (rsk)."""
            rsk = [ap.tile((SKR, N), BF16, tag=f"rsk{b}", bufs=2,
                           name=f"rsk{b}_{i}") for b in BS]
            sks = {}
            for nsi, ns in enumerate(NS):
                for b in BS:
                    xn = st[b]["xn"]
                    sk_ps = pp.tile((SKR, 512), F32, tag="pwork", bufs=4,
                                    name=f"skp{b}_{i}_{nsi}")
                    nc.tensor.matmul(sk_ps[:64], wskip[0][:, i, :],
                                     xn[0][:, ns], start=True, stop=True)
                    nc.tensor.matmul(sk_ps[64:], wskip[1][:, i, :],
                                     xn[1][:, ns], start=True, stop=True)
                    sks[b, nsi] = sk_ps
                for b in BS:
                    if b % 2 == 0:
                        nc.vector.tensor_scalar(rsk[b][:, ns], sks[b, nsi][:],
                                                vcol(f"skb{i}", SKR), 0.0,
                                                ALU.add, ALU.max)
                    else:
                        nc.scalar.activation(rsk[b][:, ns], sks[b, nsi][:],
                                             AF.Relu, bias=vcol(f"skb{i}", SKR))
            return rsk

        def eps_part(i, rsk):
            """end1 matmul on rsk; both batches stacked on the partition dim
            of one PSUM tile so a single DVE op accumulates them (cost is
            free-size-bound, partitions are parallel)."""
            for nsi, ns in enumerate(NS):
                e_ps = pp.tile((128, 512), F32, tag="pwork", bufs=4,
                               name=f"eps_{i}_{nsi}")
                for b in BS:
                    nc.tensor.matmul(e_ps[64 * b:64 * (b + 1), :],
                                     we1[:, i, :], rsk[b][:, ns],
                                     start=True, stop=True)
                if i == 0:
                    nc.vector.tensor_copy(ends[:, ns], e_ps[:])
                else:
                    nc.vector.scalar_tensor_tensor(
                        ends[:, ns], e_ps[:], 0.0,
                        ends[:, ns], ALU.bypass, ALU.add)

        def prenxs(i):
            for c in range(2):
                rows = CH[c][1]
                nxs = [ap.tile((rows, N), F32, tag=f"tmp{b}_{c}",
                               name=f"nxs{b}_{i}_{c}") for b in BS]
                for nsi, ns in enumerate(NS):
                    for b in BS:
                        xt = st_xt(b)
                        nc.gpsimd.tensor_scalar(
                            nxs[b][:, ns], xt[c][:, ns].bitcast(F32),
                            vcol(f"bns{i}_{c}", rows),
                            vcol(f"bnb{i}_{c}", rows), ALU.mult, ALU.add)
                for b in BS:
                    st[b].setdefault("nxs", [None, None])[c] = nxs[b]

        def gconv_unit(i, c, nsi):
            rows, ns = CH[c][1], NS[nsi]
            if nsi == 0:
                for b in BS:
                    st[b].setdefault("nxt", [None, None])[c] = ap.tile(
                        (rows, N), F32R, tag=f"XT{b}_{c}", bufs=2,
                        name=f"XT{b}_{i}_{c}")
            gps = []
            for b in BS:
                xn = st[b]["xn"]
                g_ps = pp.tile((rows, 512), F32, tag="pg", bufs=2,
                               name=f"gp{b}_{i}_{c}_{nsi}")
                nc.tensor.matmul(g_ps[:], gcw0[c][:, i, :],
                                 xn[c][:, ns], start=True, stop=False)
                nc.tensor.matmul(g_ps[:], wh1a[c][:, i, :, :],
                                 php[b][c][:, :, ns], perf_mode=DR,
                                 start=False, stop=False)
                nc.tensor.matmul(g_ps[:], wg12[c][:, i, :],
                                 h2t[b][c][:, ns],
                                 start=False, stop=True)
                gps.append(g_ps)
            for b in BS:
                nc.vector.scalar_tensor_tensor(
                    st[b]["nxt"][c][:, ns], gps[b][:],
                    vcol(f"bnsg{i}_{c}", rows), st[b]["nxs"][c][:, ns],
                    ALU.mult, ALU.add)
            if nsi == 1:
                for b in BS:
                    st_xt(b)[c] = st[b]["nxt"][c]

        # ---------------- end convs        # ---------------- end convs ----------------
        def tail():
            o1m = ap.tile((128, N), F32R, tag="o1", name="o1m")
            obs = {b: ap.tile((12, N), F32, tag=f"ob{b}", name=f"ob{b}")
                   for b in BS}
            # ends is batch-stacked and e1b/we2t are already duplicated
            # across both partition halves: one relu covers both batches.
            for nsi, ns in enumerate(NS):
                nc.scalar.activation(o1m[:, ns], ends[:, ns],
                                     AF.Relu, bias=vcol("e1b", 128))
            for nsi, ns in enumerate(NS):
                for b in BS:
                    o2_ps = pp.tile((12, 512), F32, tag="pwork", bufs=4,
                                    name=f"o2p{b}_{nsi}")
                    nc.tensor.matmul(o2_ps[:],
                                     we2t[64 * b:64 * (b + 1), :],
                                     o1m[64 * b:64 * (b + 1), ns],
                                     start=True, stop=True)
                    nc.vector.tensor_scalar(obs[b][:, ns], o2_ps[:],
                                            vcol("e2b", 12), None,
                                            ALU.add)
                    nc.sync.dma_start(out=outp[b][:, ns], in_=obs[b][:, ns])

        # ---------------- emission ----------------
        # Dummy sigmoid as the very first Act op: pins the
        # sigmoid_and_others activation table (which covers every function
        # this kernel uses) so only one table load is ever issued, and it
        # happens during the initial DMA wait.
        actwarm = ap.tile((1, 8), F32, tag="actwarm", name="actwarm")
        nc.vector.memset(actwarm[:], 0.0)
        nc.scalar.activation(actwarm[:], actwarm[:], AF.Sigmoid)
        phase0_pair()
        for b in BS:
            for c in range(2):
                # plane 1 only needs finite contents (stationary is zero);
                # deferred past phase0_pair so its DVE evictions go first
                nc.vector.memset(r1p[b][c][:, 1, :], 0.0)
        att(0)
        phase0_D(fillers=[lambda: tpx(0, 0), lambda: tpx(0, 1)])
        for i in range(L):
            if i == L - 1:
                # the trunk/hop outputs of the last layer are dead code:
                # only its attention + skip contribution reach the output
                for b in BS:
                    st[b]["xn"] = st[b]["xn_next"]
                eps_part(i, skp_part(i))
                break
            if i > 0:
                for b in BS:
                    tpx(i, b)
            prenxs(i)
            for p, b in ((0, 0), (1, 0), (0, 1), (2, 0),
                         (1, 1), (3, 0), (2, 1), (3, 1)):
                hop1v_pb(i, p, b)
            rsk = skp_part(i)
            eps_part(i, rsk)
            for c in range(2):
                for nsi in range(2):
                    hop1t_unit(i, c, nsi)
            hop2t_unit(i, 0, 0)
            hop2t_unit(i, 0, 1)
            gconv_unit(i, 0, 0)
            gconv_unit(i, 0, 1)
            hop2t_unit(i, 1, 0)
            hop2t_unit(i, 1, 1)
            att_c(i + 1, 0)
            gconv_unit(i, 1, 0)
            gconv_unit(i, 1, 1)
            att_c(i + 1, 1)
        tail()

    nc.finalize()
    return nc


# ----------------------------------------------------------------------------
# host-side preprocessing
# ----------------------------------------------------------------------------

def _prep_host(inputs):
    f = lambda x: np.asarray(x, dtype=np.float32)
    bf = lambda x: np.ascontiguousarray(x).astype(ml_dtypes.bfloat16)
    f8 = lambda x: np.ascontiguousarray(x).astype(ml_dtypes.float8_e4m3)
    f85 = lambda x: np.ascontiguousarray(x).astype(ml_dtypes.float8_e5m2)
    x_in = f(inputs["inputs"])
    ind = np.asarray(inputs["ind"]).astype(np.int64)
    p1, p2, p3, pk = f(inputs["p1"]), f(inputs["p2"]), f(inputs["p3"]), f(inputs["pk"])

    xo = np.pad(x_in, ((0, 0), (0, 0), (0, 0), (RF - T, 0)))
    xo_t = xo.transpose(0, 1, 3, 2)               # (B, 2, RF, N)
    te = p1[ind]
    adp = np.einsum("bi,ijk->bjk", te, pk).astype(np.float32)

    start_w, start_b = f(inputs["start_w"]), f(inputs["start_b"])
    starta_w, starta_b = f(inputs["starta_w"]), f(inputs["starta_b"])
    fc1_w, fc2_w = f(inputs["fc1_w"]), f(inputs["fc2_w"])
    skip_w, skip_b = f(inputs["skip_w"]), f(inputs["skip_b"])
    gconv_w, gconv_b = f(inputs["gconv_w"]), f(inputs["gconv_b"])
    bn_g, bn_b = f(inputs["bn_g"]), f(inputs["bn_b"])
    bna_g, bna_b = f(inputs["bna_g"]), f(inputs["bna_b"])
    end1_w, end1_b = f(inputs["end1_w"]), f(inputs["end1_b"])
    end2_w, end2_b = f(inputs["end2_w"]), f(inputs["end2_b"])

    # start convs on host: l-major T-layout rows (l*16+ch)
    x0 = (start_w[:, 0][None, None, :, None] * xo_t[:, 0][:, :, None, :]
          + start_b[None, None, :, None]).reshape(B, CL, N)
    xa = (starta_w[:, 0][None, None, :, None] * xo_t[:, 1][:, :, None, :]
          + starta_b[None, None, :, None]).reshape(B, CL, N)
    xa8 = f8(xa)

    e8, e5 = np.eye(8, dtype=np.float32), np.eye(5, dtype=np.float32)
    kr = lambda e, w: np.kron(e, np.ascontiguousarray(w.T)).astype(np.float32)

    bns = (bn_g / np.sqrt(1.0 + BN_EPS)).astype(np.float32)
    bnas = (bna_g / np.sqrt(1.0 + BN_EPS)).astype(np.float32)

    # per-layer xa scale av and folded bias bv
    avs, bvs = [np.ones(16, dtype=np.float32)], [np.zeros(16, dtype=np.float32)]
    for i in range(L):
        avs.append(2.0 * bnas[i] * avs[i])
        bvs.append(2.0 * bnas[i] * bvs[i] + bna_b[i])

    gcw0_c, wg12_c, wh1a_c, wskip_c, wfc2p_c = [], [], [], [], []
    for c, (e, rows, reps) in enumerate(((e8, 128, 8), (e5, 80, 5))):
        f2 = np.stack([kr(e, fc2_w) / SR,
                       np.zeros((rows, rows), dtype=np.float32)], axis=1)
        wfc2p_c.append(f8(f2.reshape(rows, 2 * rows)))
        g0 = np.stack([kr(e, gconv_w[i][:, 0:16]) for i in range(L)],
                      axis=1)
        g1 = np.stack([kr(e, gconv_w[i][:, 16:32]) / S1
                      for i in range(L)], axis=1)
        g2 = np.stack([kr(e, gconv_w[i][:, 32:48]) / S1
                       for i in range(L)], axis=1)
        wavm = np.stack([np.diag(np.tile(avs[i], reps))
                         for i in range(L)], axis=1)   # (rows, L, rows)
        wh = np.stack([g1, wavm], axis=2)              # (rows, L, 2, rows)
        wsk = np.stack([kr(e, skip_w[i]) for i in range(L)], axis=1)
        gcw0_c.append(bf(g0.reshape(rows, L * rows)))
        wg12_c.append(bf(g2.reshape(rows, L * rows)))
        wh1a_c.append(f8(wh.reshape(rows, L * 2 * rows)))
        wskip_c.append(bf(wsk.reshape(rows, L * (64, 40)[c])))

    # end1 columns: ref skip rows are o*13+l within the (L-1-i)-th block;
    # ours are l*8+o
    we1 = np.zeros((SKR, L, 64), dtype=np.float32)
    ll, oo = np.meshgrid(np.arange(RF), np.arange(SC), indexing="ij")
    src_col = oo.ravel() * RF + ll.ravel()
    for i in range(L):
        we1[:, i, :] = end1_w[:, (L - 1 - i) * SKR + src_col].T

    t8 = lambda v: np.tile(v, 8)
    vecs = np.zeros((128, NV_COLS), dtype=np.float32)
    ci = 0
    for i in range(L):
        vecs[:SKR, ci] = np.tile(skip_b[i], RF); ci += 1
    for i in range(L):
        bnb_adj = bn_b[i] + bns[i] * (gconv_b[i] + bvs[i])
        vecs[:, ci] = t8(bns[i]); ci += 1
        vecs[:, ci] = t8(bnb_adj); ci += 1
        vecs[:, ci] = t8(GM * bns[i]); ci += 1
        vecs[:80, ci] = np.tile(bns[i], 5); ci += 1
        vecs[:80, ci] = np.tile(bnb_adj, 5); ci += 1
        vecs[:80, ci] = np.tile(GM * bns[i], 5); ci += 1
    vecs[:64, ci] = end1_b
    vecs[64:128, ci] = end1_b; ci += 1
    vecs[:12, ci] = end2_b; ci += 1
    vecs[:, ci] = -20.0; ci += 1
    assert ci == NV_COLS

    shared = {
        "wfp": np.concatenate(
            [kr(e8, fc1_w),
             np.pad(kr(e5, fc1_w), ((0, 48), (0, 0))),
             0.5 * np.eye(128, dtype=np.float32)], axis=1),
        "idenb": np.eye(128, dtype=ml_dtypes.bfloat16),
        "gcw0_0": gcw0_c[0], "gcw0_1": gcw0_c[1],
        "wg12_0": wg12_c[0], "wg12_1": wg12_c[1],
        "wh1a_0": wh1a_c[0], "wh1a_1": wh1a_c[1],
        "wskip_0": wskip_c[0], "wskip_1": wskip_c[1],
        "wfc2p_0": wfc2p_c[0], "wfc2p_1": wfc2p_c[1],
        "we1": bf(we1.reshape(SKR, L * 64)),
        "we2t": np.concatenate([end2_w.T, end2_w.T], axis=0).astype(np.float32),
        "vecs": vecs,
    }
    in_maps = []
    for cix in range(NCORES):
        bs = slice(cix * BPC, (cix + 1) * BPC)
        m = dict(shared)
        m["x0c0"] = np.ascontiguousarray(x0[bs, 0:128])
        m["x0c1"] = np.ascontiguousarray(x0[bs, 128:208])
        m["xa8c0"] = np.ascontiguousarray(xa8[bs, 0:128])
        m["xa8c1"] = np.ascontiguousarray(xa8[bs, 128:208])
        m["ph0"] = np.ascontiguousarray(np.concatenate(
            [p2.T, p3[:DIMS, :DIMS].T,
             adp[bs].transpose(1, 0, 2).reshape(DIMS, BPC * DIMS)], axis=1))
        in_maps.append(m)
    return in_maps


def _get_nc():
    global _CACHED
    if _CACHED is None:
        _CACHED = _build_nc()
    return _CACHED


def run(inputs, trace=False):
    nc = _get_nc()
    in_maps = _prep_host(inputs)
    res = run_bass_kernel_spmd(nc, in_maps, core_ids=list(range(NCORES)),
                               trace=trace)
    out = np.stack([res.results[c]["outp"] for c in range(NCORES)])
    out = out.reshape(B, 12, N, 1).astype(np.float32)
    return out, res


def kernel(**inputs):
    out, _ = run(inputs)
    return out



# revision 77
# speedup vs baseline: 1.0087x; 1.0087x over previous
"""DMSTGCN forward on 8 Trainium2 NeuronCores (Bass/Tile).

Sharding: data-parallel over batch B=16 -> 2 batches per core; parameters
replicated. The dynamic adjacency (1024x1024 per batch) is built on device
(clamp evictions split DVE/Act, the Act half as a sigmoid step) and held in
SBUF as fp8e4 in a DoubleRow K-pair layout (128, 2, N); all three hop matmul
groups run as fp8 DoubleRow (0.5 cycles/row, 256-deep contraction per
instruction). Hop1 produces node-major (V) output that hop2 consumes directly
as a DoubleRow stationary, emitting channel-major (T) output - so no PE
back-transposes are needed. The gconv consumes an fp8 (h1*S1, xa) pair via
one DoubleRow matmul (folding the per-layer xa diagonal) plus bf16 matmuls
for the xn and h2 terms; h2 stays bf16 because its fp8 quantization breaks
the error budget (h2max*W2max > 240^2). The attention fc2 also runs as an
fp8 DoubleRow pair over (r1/256, zero-weighted filler). Start convs are
computed on the host and shipped; phase0-critical inputs are packed into
single DMAs (each DMA carries ~2.2us fixed latency). end1 skip
contributions accumulate into a batch-stacked (128, N) SBUF tensor: the
end1 matmuls write both batches into partition halves of one PSUM tile so
a single DVE op per column-half accumulates them. Trunk math stays float32r (fp8/bf16 trunk compounds ~3%/layer through
the sigmoid boundary and is not safe). Eviction engines are hand-balanced
between DVE and Act; the tail runs all o1 on Act because DVE is backlogged
with the layer-7 end1 accumulates.
"""
import numpy as np
import ml_dtypes

import concourse.bacc as bacc
import concourse.mybir as mybir
from concourse.tile import TileContext
from concourse.bass_utils import run_bass_kernel_spmd

F32 = mybir.dt.float32
F32R = mybir.dt.float32r
BF16 = mybir.dt.bfloat16
F8 = mybir.dt.float8e4
F8E5 = mybir.dt.float8e5
AF = mybir.ActivationFunctionType
ALU = mybir.AluOpType
DR = mybir.MatmulPerfMode.DoubleRow

B, N, T, RF = 16, 1024, 12, 13
RC, SC, DIMS, L = 16, 8, 32, 8
BN_EPS = 1e-5
NCORES = 8
BPC = B // NCORES          # batches per core
CL = RC * RF               # 208 rows in T-layout
SKR = SC * RF              # 104 skip rows
CH = ((0, 128), (128, 80))  # l-major T-layout row chunks
CLS = (slice(0, 128), slice(128, 208))  # CL slices per chunk
NV_COLS = L + L * 2 * 3 + 3
S1 = 1.0 / 32.0            # h1 storage scale (fp8)
SR = 1.0 / 256.0           # r1 storage scale (fp8)
S2 = 1.0 / 64.0            # extra h2 eviction scale (net h2/2048)
GM = 1.0                   # gconv PSUM scale (h2 stays bf16)

_CACHED = None


def _build_nc():
    nc = bacc.Bacc("TRN2", target_bir_lowering=False)

    d = {}
    def din(name, shape, dt=F32R):
        d[name] = nc.dram_tensor(name, list(shape), dt, kind="ExternalInput")

    din("x0c0", (BPC, 128, N))
    din("x0c1", (BPC, 80, N))
    din("xa8c0", (BPC, 128, N), F8)
    din("xa8c1", (BPC, 80, N), F8)
    din("ph0", (DIMS, N + DIMS + BPC * DIMS))
    din("wfp", (128, 336))
    din("idenb", (128, 128), BF16)
    din("gcw0_0", (128, L * 128), BF16)
    din("gcw0_1", (80, L * 80), BF16)
    din("wg12_0", (128, L * 128), BF16)
    din("wg12_1", (80, L * 80), BF16)
    din("wh1a_0", (128, L * 2 * 128), F8)
    din("wh1a_1", (80, L * 2 * 80), F8)
    din("wfc2p_0", (128, 2 * 128), F8)
    din("wfc2p_1", (80, 2 * 80), F8)
    din("wskip_0", (128, L * 64), BF16)
    din("wskip_1", (80, L * 40), BF16)
    din("we1", (SKR, L * 64), BF16)
    din("we2t", (128, 12))
    din("vecs", (128, NV_COLS), F32)
    outp = nc.dram_tensor("outp", [BPC, 12, N], F32, kind="ExternalOutput")

    with TileContext(nc) as tc, \
         tc.tile_pool(name="wp", bufs=1) as wp, \
         tc.tile_pool(name="ap", bufs=1) as ap, \
         tc.tile_pool(name="pp", bufs=1, space="PSUM") as pp:

        def wtile(name, src_ap, shape, dt=F32R, eng=None):
            t = wp.tile(shape, dt, tag=name, name=name)
            (eng or nc.sync).dma_start(out=t[:], in_=src_ap)
            return t

        # phase0-critical loads packed into ONE DMA on the SP queue (each
        # DMA carries ~2.2us fixed latency); bulk weights on gpsimd.
        ph0 = wtile("ph0", d["ph0"][:], (DIMS, N + DIMS + BPC * DIMS),
                    eng=nc.sync)
        p2T = ph0[:, 0:N]
        p3sT = ph0[:, N:N + DIMS]
        adps = [ph0[:, N + DIMS + b * DIMS:N + DIMS + (b + 1) * DIMS]
                for b in range(BPC)]

        # trunk/attention tiles; x0 DMA'd straight into the first XT
        # buffers, with the att weights interleaved so att(0, b0) can start
        # before batch 1's x0 lands.
        xts = [[None, None] for _ in range(BPC)]
        for b in range(BPC):
            for c in range(2):
                rows = CH[c][1]
                xts[b][c] = ap.tile((rows, N), F32R, tag=f"XT{b}_{c}",
                                    bufs=2, name=f"XT{b}_{c}_init")
        nc.sync.dma_start(out=xts[0][0][:], in_=d["x0c0"][0])
        wfp = wtile("wfp", d["wfp"][:], (128, 336))
        wfc1 = [wfp[:, 0:128], wfp[:80, 128:208]]
        idenh = wfp[:, 208:336]
        nc.sync.dma_start(out=xts[0][1][:], in_=d["x0c1"][0])
        nc.sync.dma_start(out=xts[1][0][:], in_=d["x0c0"][1])
        nc.sync.dma_start(out=xts[1][1][:], in_=d["x0c1"][1])
        idenb = wtile("idenb", d["idenb"][:], (128, 128), BF16)
        vecs = wtile("vecs", d["vecs"][:], (128, NV_COLS), F32)

        php = [[ap.tile((CH[c][1], 2, N), F8, tag=f"PH{b}_{c}",
                        name=f"PH{b}_{c}") for c in range(2)]
               for b in range(BPC)]
        r1p = [[ap.tile((CH[c][1], 2, N), F8, tag=f"R1{b}_{c}",
                        name=f"R1{b}_{c}") for c in range(2)]
               for b in range(BPC)]

        h2t = [[ap.tile((CH[c][1], N), BF16, tag=f"H2{b}_{c}",
                        name=f"H2{b}_{c}") for c in range(2)]
               for b in range(BPC)]
        wfc2p = [wtile(f"wfc2p_{c}", d[f"wfc2p_{c}"][:],
                       (CH[c][1], 2, CH[c][1]), F8, eng=nc.gpsimd)
                 for c in range(2)]
        for b in range(BPC):
            for c in range(2):
                nc.gpsimd.dma_start(out=php[b][c][:, 1, :],
                                    in_=d[f"xa8c{c}"][b])
        gcw0 = [wtile(f"gcw0_{c}", d[f"gcw0_{c}"][:],
                      (CH[c][1], L, CH[c][1]), BF16, eng=nc.gpsimd)
                for c in range(2)]
        wg12 = [wtile(f"wg12_{c}", d[f"wg12_{c}"][:],
                      (CH[c][1], L, CH[c][1]), BF16, eng=nc.gpsimd)
                for c in range(2)]
        wh1a = [wtile(f"wh1a_{c}", d[f"wh1a_{c}"][:],
                      (CH[c][1], L, 2, CH[c][1]), F8, eng=nc.gpsimd)
                for c in range(2)]
        wskip = [wtile(f"wskip_{c}", d[f"wskip_{c}"][:],
                       (CH[c][1], L, (64, 40)[c]), BF16, eng=nc.gpsimd)
                 for c in range(2)]
        we1 = wtile("we1", d["we1"][:], (SKR, L, 64), BF16, eng=nc.gpsimd)
        we2t = wtile("we2t", d["we2t"][:], (128, 12), eng=nc.gpsimd)

        vc = {}
        ci = 0
        for i in range(L):
            vc[f"skb{i}"] = ci; ci += 1
        for i in range(L):
            for c in range(2):
                for nm in ("bns", "bnb", "bnsg"):
                    vc[f"{nm}{i}_{c}"] = ci; ci += 1
        vc["e1b"] = ci; ci += 1
        vc["e2b"] = ci; ci += 1
        vc["neg20"] = ci; ci += 1
        assert ci == NV_COLS

        def vcol(nm, rows=128):
            return vecs[:rows, vc[nm]:vc[nm] + 1]

        NS = (slice(0, 512), slice(512, 1024))
        BS = range(BPC)

        st = [dict() for _ in range(BPC)]
        # persistent fp8 adjacency in DoubleRow pair layout
        ATd = [[ap.tile((128, 2, N), F8, tag=f"ATd{b}_{k}", name=f"ATd{b}_{k}")
                for k in range(4)] for b in BS]
        ends = ap.tile((128, N), F32, tag="END", name="END")

        # PE p-state warmup: tiny matmuls as early as possible so the
        # 3us ramp window elapses before the heavy phase0 matmuls.
        def warmup():
            wps = pp.tile((DIMS, 8), F32, tag="pwork", bufs=4, name="warm")
            for r in range(6):
                nc.tensor.matmul(wps[:], adps[0][:], p2T[:, 0:8],
                                 start=(r == 0), stop=(r == 5))

        # ---------------- adjacency ----------------
        def phase0_pair():
            Lst = [ap.tile((64, N), F32R, tag=f"Lst{b}", name=f"Lst{b}")
                   for b in BS]
            Rst = [ap.tile((64, N), F32R, tag=f"Rst{b}", name=f"Rst{b}")
                   for b in BS]
            for nsi, ns in enumerate(NS):
                pss = []
                for b in BS:
                    ps = pp.tile((DIMS, 512), F32, tag="pwork", bufs=4,
                                 name=f"srcTps{b}_{nsi}")
                    nc.tensor.matmul(ps[:], adps[b], p2T[:, ns],
                                     start=True, stop=True)
                    pss.append(ps)
                for b in BS:
                    nc.scalar.activation(Rst[b][0:32, ns], pss[b][:], AF.Copy)
                    nc.vector.tensor_scalar(Lst[b][32:64, ns], pss[b][:],
                                            -1.0, None, ALU.mult)
            for nsi, ns in enumerate(NS):
                pss = []
                for b in BS:
                    ps = pp.tile((DIMS, 512), F32, tag="pwork", bufs=4,
                                 name=f"ups{b}_{nsi}")
                    nc.tensor.matmul(ps[:], p3sT, Rst[b][0:32, ns],
                                     start=True, stop=True)
                    pss.append(ps)
                for b in BS:
                    nc.scalar.activation(Lst[b][0:32, ns], pss[b][:], AF.Copy)
                    nc.vector.tensor_copy(Rst[b][32:64, ns], pss[b][:])
            st[0]["LR"] = (Lst, Rst)

        def phase0_D(fillers=()):
            fillers = list(fillers)
            Lst, Rst = st[0]["LR"]
            for v in range(8):
                cs = slice(v * 128, (v + 1) * 128)
                for nsi, ns in enumerate(NS):
                    dpss = []
                    for b in BS:
                        dps = pp.tile((128, 512), F32, tag="pwork", bufs=4,
                                      name=f"dps{b}_{v}_{nsi}")
                        nc.tensor.matmul(dps[:], Lst[b][:, cs], Rst[b][:, ns],
                                         start=True, stop=True)
                        dpss.append(dps)
                    # A = clamp(D, 0, 1): |D| ~ 5e4 >> 1 and off-diagonal
                    # |D| >= 0.03, so clamp == step(D>0) except on a ~2e-5
                    # sliver; diag D == 0 maps to 0 via the -20 bias. Act
                    # takes half the evictions as sigmoid(1e6 D - 20).
                    for b in BS:
                        if (b + v + nsi) % 2 == 0:
                            nc.vector.tensor_scalar(
                                ATd[b][v // 2][:, v % 2, ns], dpss[b][:],
                                0.0, 1.0, ALU.max, ALU.min)
                        else:
                            nc.scalar.activation(
                                ATd[b][v // 2][:, v % 2, ns], dpss[b][:],
                                AF.Sigmoid, bias=vcol("neg20"),
                                scale=1e6)
                if v % 2 == 1 and fillers:
                    fillers.pop(0)()
            while fillers:
                fillers.pop(0)()

        # ---------------- layer stages ----------------
        def att_c(i, c):
            """fc1 -> relu -> fc2 + x/2 -> sigmoid(2.) for one chunk."""
            rows = CH[c][1]
            xn = {}
            for b in BS:
                if c == 0:
                    st[b]["xn_next"] = [None, None]
                xn[b] = ap.tile((rows, N), BF16, tag=f"XN{b}_{c}",
                                bufs=2, name=f"XN{b}_{i}_{c}")
                st[b]["xn_next"][c] = xn[b]
            m1s, apss = {}, {}
            for nsi in range(2):
                ns = NS[nsi]
                for b in BS:
                    xt = st_xt(b)
                    m1 = pp.tile((rows, 512), F32, tag="pwork", bufs=4,
                                 name=f"m1_{b}_{i}_{c}_{nsi}")
                    nc.tensor.matmul(m1[:], wfc1[c], xt[c][:, ns],
                                     start=True, stop=True)
                    m1s[b, nsi] = m1
                for b in BS:
                    if b % 2 == 0:
                        nc.scalar.activation(r1p[b][c][:, 0, ns],
                                             m1s[b, nsi][:],
                                             AF.Relu, scale=SR)
                    else:
                        nc.vector.tensor_scalar(r1p[b][c][:, 0, ns],
                                                m1s[b, nsi][:],
                                                SR, 0.0,
                                                ALU.mult, ALU.max)
            for nsi in range(2):
                ns = NS[nsi]
                for b in BS:
                    xt = st_xt(b)
                    a_ps = pp.tile((rows, 512), F32, tag="pwork", bufs=4,
                                   name=f"aps{b}_{i}_{c}_{nsi}")
                    nc.tensor.matmul(a_ps[:], wfc2p[c][:, :, :],
                                     r1p[b][c][:, :, ns], perf_mode=DR,
                                     start=True, stop=False)
                    nc.tensor.matmul(a_ps[:], idenh[:rows, 0:rows],
                                     xt[c][:, ns], start=False, stop=True)
                    apss[b, nsi] = a_ps
                for b in BS:
                    nc.scalar.activation(xn[b][:, ns], apss[b, nsi][:],
                                         AF.Sigmoid, scale=2.0)

        def att(i):
            att_c(i, 0)
            att_c(i, 1)

        def st_xt(b):
            if "xt" not in st[b]:
                st[b]["xt"] = [xts[b][0], xts[b][1]]
            return st[b]["xt"]

        def tpx(i, b):
            """xn -> V-layout fp8 DoubleRow pairs xvd[kp] (128,2,CL)."""
            st[b]["xn"] = st[b]["xn_next"]
            xn = st[b]["xn"]
            xvd = [None] * 4
            for kp in range(4):
                tp = pp.tile((128, 2, CL), BF16, tag="ptr", bufs=2,
                             name=f"tpx{b}_{i}_{kp}")
                for s in range(2):
                    v = 2 * kp + s
                    cs = slice(v * 128, (v + 1) * 128)
                    for c in range(2):
                        o, rows = CH[c]
                        nc.tensor.transpose(tp[:, s, o:o + rows],
                                            xn[c][:, cs], idenb[:rows, :rows])
                xvd[kp] = ap.tile((128, 2, CL), F8, tag=f"XV{b}_{kp}",
                                  bufs=2, name=f"XV{b}_{i}_{kp}")
                if kp % 2 == 0:
                    nc.vector.tensor_copy(xvd[kp][:], tp[:])
                else:
                    nc.scalar.activation(xvd[kp][:], tp[:], AF.Copy)
            st[b]["xvd"] = xvd

        def hop1v_pb(i, p, b):
            """h1 V-pair for one w-pair p, one batch."""
            if p == 0:
                st[b]["h1d"] = [None] * 4
            xvd = st[b]["xvd"]
            h_ps = pp.tile((128, 2, CL), F32, tag="ptr", bufs=2,
                           name=f"hp{b}_{i}_{p}")
            for s in range(2):
                w = 2 * p + s
                ws = slice(w * 128, (w + 1) * 128)
                dst = h_ps[:, s, :]
                for kp in range(4):
                    nc.tensor.matmul(dst, ATd[b][kp][:, :, ws],
                                     xvd[kp][:], perf_mode=DR,
                                     start=(kp == 0), stop=(kp == 3))
            t = ap.tile((128, 2, CL), F8, tag=f"H1{b}_{p}",
                        bufs=2, name=f"H1{b}_{i}_{p}")
            if p < 3:
                nc.scalar.activation(t[:], h_ps[:], AF.Identity,
                                     scale=S1)
            else:
                nc.vector.tensor_scalar(t[:], h_ps[:], S1, None,
                                        ALU.mult)
            st[b]["h1d"][p] = t

        def hop1t_unit(i, c, nsi):
            rows, ns = CH[c][1], NS[nsi]
            g1s = {}
            for b in BS:
                xvd = st[b]["xvd"]
                g1 = pp.tile((rows, 512), F32, tag="pwork", bufs=4,
                             name=f"g1_{b}_{i}_{c}_{nsi}")
                for kp in range(4):
                    nc.tensor.matmul(g1[:], xvd[kp][:, :, CLS[c]],
                                     ATd[b][kp][:, :, ns], perf_mode=DR,
                                     start=(kp == 0), stop=(kp == 3))
                g1s[b] = g1
            for b in BS:
                dst = php[b][c][:, 0, ns]
                if (b + c + nsi) % 2 == 0:
                    nc.scalar.activation(dst, g1s[b][:], AF.Identity,
                                         scale=S1)
                else:
                    nc.vector.tensor_scalar(dst, g1s[b][:], S1, None,
                                            ALU.mult)

        def hop2t_unit(i, c, nsi):
            rows, ns = CH[c][1], NS[nsi]
            g2s = {}
            for b in BS:
                h1d = st[b]["h1d"]
                g2 = pp.tile((rows, 512), F32, tag="pwork", bufs=4,
                             name=f"g2_{b}_{i}_{c}_{nsi}")
                for kp in range(4):
                    nc.tensor.matmul(g2[:], h1d[kp][:, :, CLS[c]],
                                     ATd[b][kp][:, :, ns], perf_mode=DR,
                                     start=(kp == 0), stop=(kp == 3))
                g2s[b] = g2
            for b in BS:
                dst = h2t[b][c][:, ns]
                if (b + c + nsi) % 2 == 1:
                    nc.scalar.activation(dst, g2s[b][:], AF.Copy)
                else:
                    nc.vector.tensor_copy(dst, g2s[b][:])

        def skp_part(i):
            """skip conv -> relu (rsk)."""
            rsk = [ap.tile((SKR, N), BF16, tag=f"rsk{b}", bufs=2,
                           name=f"rsk{b}_{i}") for b in BS]
            sks = {}
            for nsi, ns in enumerate(NS):
                for b in BS:
                    xn = st[b]["xn"]
                    sk_ps = pp.tile((SKR, 512), F32, tag="pwork", bufs=4,
                                    name=f"skp{b}_{i}_{nsi}")
                    nc.tensor.matmul(sk_ps[:64], wskip[0][:, i, :],
                                     xn[0][:, ns], start=True, stop=True)
                    nc.tensor.matmul(sk_ps[64:], wskip[1][:, i, :],
                                     xn[1][:, ns], start=True, stop=True)
                    sks[b, nsi] = sk_ps
                for b in BS:
                    if b % 2 == 0:
                        nc.vector.tensor_scalar(rsk[b][:, ns], sks[b, nsi][:],
                                                vcol(f"skb{i}", SKR), 0.0,
                                                ALU.add, ALU.max)
                    else:
                        nc.scalar.activation(rsk[b][:, ns], sks[b, nsi][:],
                                             AF.Relu, bias=vcol(f"skb{i}", SKR))
            return rsk

        def eps_part(i, rsk):
            """end1 matmul on rsk; both batches stacked on the partition dim
            of one PSUM tile so a single DVE op accumulates them (cost is
            free-size-bound, partitions are parallel)."""
            for nsi, ns in enumerate(NS):
                e_ps = pp.tile((128, 512), F32, tag="pwork", bufs=4,
                               name=f"eps_{i}_{nsi}")
                for b in BS:
                    nc.tensor.matmul(e_ps[64 * b:64 * (b + 1), :],
                                     we1[:, i, :], rsk[b][:, ns],
                                     start=True, stop=True)
                if i == 0:
                    nc.vector.tensor_copy(ends[:, ns], e_ps[:])
                else:
                    nc.vector.scalar_tensor_tensor(
                        ends[:, ns], e_ps[:], 0.0,
                        ends[:, ns], ALU.bypass, ALU.add)

        def prenxs(i):
            for c in range(2):
                rows = CH[c][1]
                nxs = [ap.tile((rows, N), F32, tag=f"tmp{b}_{c}",
                               name=f"nxs{b}_{i}_{c}") for b in BS]
                for nsi, ns in enumerate(NS):
                    for b in BS:
                        xt = st_xt(b)
                        nc.gpsimd.tensor_scalar(
                            nxs[b][:, ns], xt[c][:, ns].bitcast(F32),
                            vcol(f"bns{i}_{c}", rows),
                            vcol(f"bnb{i}_{c}", rows), ALU.mult, ALU.add)
                for b in BS:
                    st[b].setdefault("nxs", [None, None])[c] = nxs[b]

        def gconv_unit(i, c, nsi):
            rows, ns = CH[c][1], NS[nsi]
            if nsi == 0:
                for b in BS:
                    st[b].setdefault("nxt", [None, None])[c] = ap.tile(
                        (rows, N), F32R, tag=f"XT{b}_{c}", bufs=2,
                        name=f"XT{b}_{i}_{c}")
            gps = []
            for b in BS:
                xn = st[b]["xn"]
                g_ps = pp.tile((rows, 512), F32, tag="pg", bufs=2,
                               name=f"gp{b}_{i}_{c}_{nsi}")
                nc.tensor.matmul(g_ps[:], gcw0[c][:, i, :],
                                 xn[c][:, ns], start=True, stop=False)
                nc.tensor.matmul(g_ps[:], wh1a[c][:, i, :, :],
                                 php[b][c][:, :, ns], perf_mode=DR,
                                 start=False, stop=False)
                nc.tensor.matmul(g_ps[:], wg12[c][:, i, :],
                                 h2t[b][c][:, ns],
                                 start=False, stop=True)
                gps.append(g_ps)
            for b in BS:
                nc.vector.scalar_tensor_tensor(
                    st[b]["nxt"][c][:, ns], gps[b][:],
                    vcol(f"bnsg{i}_{c}", rows), st[b]["nxs"][c][:, ns],
                    ALU.mult, ALU.add)
            if nsi == 1:
                for b in BS:
                    st_xt(b)[c] = st[b]["nxt"][c]

        # ---------------- end convs        # ---------------- end convs ----------------
        def tail():
            o1m = ap.tile((128, N), F32R, tag="o1", name="o1m")
            obs = {b: ap.tile((12, N), F32, tag=f"ob{b}", name=f"ob{b}")
                   for b in BS}
            # ends is batch-stacked and e1b/we2t are already duplicated
            # across both partition halves: one relu covers both batches.
            for nsi, ns in enumerate(NS):
                nc.scalar.activation(o1m[:, ns], ends[:, ns],
                                     AF.Relu, bias=vcol("e1b", 128))
            for nsi, ns in enumerate(NS):
                for b in BS:
                    o2_ps = pp.tile((12, 512), F32, tag="pwork", bufs=4,
                                    name=f"o2p{b}_{nsi}")
                    nc.tensor.matmul(o2_ps[:],
                                     we2t[64 * b:64 * (b + 1), :],
                                     o1m[64 * b:64 * (b + 1), ns],
                                     start=True, stop=True)
                    nc.vector.tensor_scalar(obs[b][:, ns], o2_ps[:],
                                            vcol("e2b", 12), None,
                                            ALU.add)
                    nc.sync.dma_start(out=outp[b][:, ns], in_=obs[b][:, ns])

        # ---------------- emission ----------------
        # Dummy sigmoid as the very first Act op: pins the
        # sigmoid_and_others activation table (which covers every function
        # this kernel uses) so only one table load is ever issued, and it
        # happens during the initial DMA wait.
        actwarm = ap.tile((1, 8), F32, tag="actwarm", name="actwarm")
        nc.vector.memset(actwarm[:], 0.0)
        nc.scalar.activation(actwarm[:], actwarm[:], AF.Sigmoid)
        phase0_pair()
        for b in BS:
            for c in range(2):
                # plane 1 only needs finite contents (stationary is zero);
                # deferred past phase0_pair so its DVE evictions go first
                nc.vector.memset(r1p[b][c][:, 1, :], 0.0)
        att(0)
        phase0_D(fillers=[lambda: tpx(0, 0), lambda: tpx(0, 1)])
        for i in range(L):
            if i == L - 1:
                # the trunk/hop outputs of the last layer are dead code:
                # only its attention + skip contribution reach the output
                for b in BS:
                    st[b]["xn"] = st[b]["xn_next"]
                eps_part(i, skp_part(i))
                break
            if i > 0:
                for b in BS:
                    tpx(i, b)
            prenxs(i)
            for p, b in ((0, 0), (1, 0), (0, 1), (2, 0),
                         (1, 1), (3, 0), (2, 1), (3, 1)):
                hop1v_pb(i, p, b)
            rsk = skp_part(i)
            eps_part(i, rsk)
            for c in range(2):
                for nsi in range(2):
                    hop1t_unit(i, c, nsi)
            hop2t_unit(i, 0, 0)
            hop2t_unit(i, 0, 1)
            gconv_unit(i, 0, 0)
            gconv_unit(i, 0, 1)
            hop2t_unit(i, 1, 0)
            hop2t_unit(i, 1, 1)
            att_c(i + 1, 0)
            gconv_unit(i, 1, 0)
            gconv_unit(i, 1, 1)
            att_c(i + 1, 1)
        tail()

    nc.finalize()
    return nc


# ----------------------------------------------------------------------------
# host-side preprocessing
# ----------------------------------------------------------------------------

def _prep_host(inputs):
    f = lambda x: np.asarray(x, dtype=np.float32)
    bf = lambda x: np.ascontiguousarray(x).astype(ml_dtypes.bfloat16)
    f8 = lambda x: np.ascontiguousarray(x).astype(ml_dtypes.float8_e4m3)
    f85 = lambda x: np.ascontiguousarray(x).astype(ml_dtypes.float8_e5m2)
    x_in = f(inputs["inputs"])
    ind = np.asarray(inputs["ind"]).astype(np.int64)
    p1, p2, p3, pk = f(inputs["p1"]), f(inputs["p2"]), f(inputs["p3"]), f(inputs["pk"])

    xo = np.pad(x_in, ((0, 0), (0, 0), (0, 0), (RF - T, 0)))
    xo_t = xo.transpose(0, 1, 3, 2)               # (B, 2, RF, N)
    te = p1[ind]
    adp = np.einsum("bi,ijk->bjk", te, pk).astype(np.float32)

    start_w, start_b = f(inputs["start_w"]), f(inputs["start_b"])
    starta_w, starta_b = f(inputs["starta_w"]), f(inputs["starta_b"])
    fc1_w, fc2_w = f(inputs["fc1_w"]), f(inputs["fc2_w"])
    skip_w, skip_b = f(inputs["skip_w"]), f(inputs["skip_b"])
    gconv_w, gconv_b = f(inputs["gconv_w"]), f(inputs["gconv_b"])
    bn_g, bn_b = f(inputs["bn_g"]), f(inputs["bn_b"])
    bna_g, bna_b = f(inputs["bna_g"]), f(inputs["bna_b"])
    end1_w, end1_b = f(inputs["end1_w"]), f(inputs["end1_b"])
    end2_w, end2_b = f(inputs["end2_w"]), f(inputs["end2_b"])

    # start convs on host: l-major T-layout rows (l*16+ch)
    x0 = (start_w[:, 0][None, None, :, None] * xo_t[:, 0][:, :, None, :]
          + start_b[None, None, :, None]).reshape(B, CL, N)
    xa = (starta_w[:, 0][None, None, :, None] * xo_t[:, 1][:, :, None, :]
          + starta_b[None, None, :, None]).reshape(B, CL, N)
    xa8 = f8(xa)

    e8, e5 = np.eye(8, dtype=np.float32), np.eye(5, dtype=np.float32)
    kr = lambda e, w: np.kron(e, np.ascontiguousarray(w.T)).astype(np.float32)

    bns = (bn_g / np.sqrt(1.0 + BN_EPS)).astype(np.float32)
    bnas = (bna_g / np.sqrt(1.0 + BN_EPS)).astype(np.float32)

    # per-layer xa scale av and folded bias bv
    avs, bvs = [np.ones(16, dtype=np.float32)], [np.zeros(16, dtype=np.float32)]
    for i in range(L):
        avs.append(2.0 * bnas[i] * avs[i])
        bvs.append(2.0 * bnas[i] * bvs[i] + bna_b[i])

    gcw0_c, wg12_c, wh1a_c, wskip_c, wfc2p_c = [], [], [], [], []
    for c, (e, rows, reps) in enumerate(((e8, 128, 8), (e5, 80, 5))):
        f2 = np.stack([kr(e, fc2_w) / SR,
                       np.zeros((rows, rows), dtype=np.float32)], axis=1)
        wfc2p_c.append(f8(f2.reshape(rows, 2 * rows)))
        g0 = np.stack([kr(e, gconv_w[i][:, 0:16]) for i in range(L)],
                      axis=1)
        g1 = np.stack([kr(e, gconv_w[i][:, 16:32]) / S1
                      for i in range(L)], axis=1)
        g2 = np.stack([kr(e, gconv_w[i][:, 32:48]) / S1
                       for i in range(L)], axis=1)
        wavm = np.stack([np.diag(np.tile(avs[i], reps))
                         for i in range(L)], axis=1)   # (rows, L, rows)
        wh = np.stack([g1, wavm], axis=2)              # (rows, L, 2, rows)
        wsk = np.stack([kr(e, skip_w[i]) for i in range(L)], axis=1)
        gcw0_c.append(bf(g0.reshape(rows, L * rows)))
        wg12_c.append(bf(g2.reshape(rows, L * rows)))
        wh1a_c.append(f8(wh.reshape(rows, L * 2 * rows)))
        wskip_c.append(bf(wsk.reshape(rows, L * (64, 40)[c])))

    # end1 columns: ref skip rows are o*13+l within the (L-1-i)-th block;
    # ours are l*8+o
    we1 = np.zeros((SKR, L, 64), dtype=np.float32)
    ll, oo = np.meshgrid(np.arange(RF), np.arange(SC), indexing="ij")
    src_col = oo.ravel() * RF + ll.ravel()
    for i in range(L):
        we1[:, i, :] = end1_w[:, (L - 1 - i) * SKR + src_col].T

    t8 = lambda v: np.tile(v, 8)
    vecs = np.zeros((128, NV_COLS), dtype=np.float32)
    ci = 0
    for i in range(L):
        vecs[:SKR, ci] = np.tile(skip_b[i], RF); ci += 1
    for i in range(L):
        bnb_adj = bn_b[i] + bns[i] * (gconv_b[i] + bvs[i])
        vecs[:, ci] = t8(bns[i]); ci += 1
        vecs[:, ci] = t8(bnb_adj); ci += 1
        vecs[:, ci] = t8(GM * bns[i]); ci += 1
        vecs[:80, ci] = np.tile(bns[i], 5); ci += 1
        vecs[:80, ci] = np.tile(bnb_adj, 5); ci += 1
        vecs[:80, ci] = np.tile(GM * bns[i], 5); ci += 1
    vecs[:64, ci] = end1_b
    vecs[64:128, ci] = end1_b; ci += 1
    vecs[:12, ci] = end2_b; ci += 1
    vecs[:, ci] = -20.0; ci += 1
    assert ci == NV_COLS

    shared = {
        "wfp": np.concatenate(
            [kr(e8, fc1_w),
             np.pad(kr(e5, fc1_w), ((0, 48), (0, 0))),
             0.5 * np.eye(128, dtype=np.float32)], axis=1),
        "idenb": np.eye(128, dtype=ml_dtypes.bfloat16),
        "gcw0_0": gcw0_c[0], "gcw0_1": gcw0_c[1],
        "wg12_0": wg12_c[0], "wg12_1": wg12_c[1],
        "wh1a_0": wh1a_c[0], "wh1a_1": wh1a_c[1],
        "wskip_0": wskip_c[0], "wskip_1": wskip_c[1],
        "wfc2p_0": wfc2p_c[0], "wfc2p_1": wfc2p_c[1],
        "we1": bf(we1.reshape(SKR, L * 64)),
        "we2t": np.concatenate([end2_w.T, end2_w.T], axis=0).astype(np.float32),
        "vecs": vecs,
    }
    in_maps = []
    for cix in range(NCORES):
        bs = slice(cix * BPC, (cix + 1) * BPC)
        m = dict(shared)
        m["x0c0"] = np.ascontiguousarray(x0[bs, 0:128])
        m["x0c1"] = np.ascontiguousarray(x0[bs, 128:208])
        m["xa8c0"] = np.ascontiguousarray(xa8[bs, 0:128])
        m["xa8c1"] = np.ascontiguousarray(xa8[bs, 128:208])
        m["ph0"] = np.ascontiguousarray(np.concatenate(
            [p2.T, p3[:DIMS, :DIMS].T,
             adp[bs].transpose(1, 0, 2).reshape(DIMS, BPC * DIMS)], axis=1))
        in_maps.append(m)
    return in_maps


def _get_nc():
    global _CACHED
    if _CACHED is None:
        _CACHED = _build_nc()
    return _CACHED


def run(inputs, trace=False):
    nc = _get_nc()
    in_maps = _prep_host(inputs)
    res = run_bass_kernel_spmd(nc, in_maps, core_ids=list(range(NCORES)),
                               trace=trace)
    out = np.stack([res.results[c]["outp"] for c in range(NCORES)])
    out = out.reshape(B, 12, N, 1).astype(np.float32)
    return out, res


def kernel(**inputs):
    out, _ = run(inputs)
    return out



# revision 78
# speedup vs baseline: 1.0115x; 1.0028x over previous
"""DMSTGCN forward on 8 Trainium2 NeuronCores (Bass/Tile).

Sharding: data-parallel over batch B=16 -> 2 batches per core; parameters
replicated. The dynamic adjacency (1024x1024 per batch) is built on device
(clamp evictions split DVE/Act, the Act half as a sigmoid step) and held in
SBUF as fp8e4 in a DoubleRow K-pair layout (128, 2, N); all three hop matmul
groups run as fp8 DoubleRow (0.5 cycles/row, 256-deep contraction per
instruction). Hop1 produces node-major (V) output that hop2 consumes directly
as a DoubleRow stationary, emitting channel-major (T) output - so no PE
back-transposes are needed. The gconv consumes an fp8 (h1*S1, xa) pair via
one DoubleRow matmul (folding the per-layer xa diagonal) plus bf16 matmuls
for the xn and h2 terms; h2 stays bf16 because its fp8 quantization breaks
the error budget (h2max*W2max > 240^2). The attention fc2 also runs as an
fp8 DoubleRow pair over (r1/256, zero-weighted filler). Start convs are
computed on the host and shipped; phase0-critical inputs are packed into
single DMAs (each DMA carries ~2.2us fixed latency). end1 skip
contributions accumulate into a batch-stacked (128, N) SBUF tensor: the
end1 matmuls write both batches into partition halves of one PSUM tile so
a single DVE op per column-half accumulates them. Trunk math stays float32r (fp8/bf16 trunk compounds ~3%/layer through
the sigmoid boundary and is not safe). Eviction engines are hand-balanced
between DVE and Act; the tail runs all o1 on Act because DVE is backlogged
with the layer-7 end1 accumulates.
"""
import numpy as np
import ml_dtypes

import concourse.bacc as bacc
import concourse.mybir as mybir
from concourse.tile import TileContext
from concourse.bass_utils import run_bass_kernel_spmd

F32 = mybir.dt.float32
F32R = mybir.dt.float32r
BF16 = mybir.dt.bfloat16
F8 = mybir.dt.float8e4
F8E5 = mybir.dt.float8e5
AF = mybir.ActivationFunctionType
ALU = mybir.AluOpType
DR = mybir.MatmulPerfMode.DoubleRow

B, N, T, RF = 16, 1024, 12, 13
RC, SC, DIMS, L = 16, 8, 32, 8
BN_EPS = 1e-5
NCORES = 8
BPC = B // NCORES          # batches per core
CL = RC * RF               # 208 rows in T-layout
SKR = SC * RF              # 104 skip rows
CH = ((0, 128), (128, 80))  # l-major T-layout row chunks
CLS = (slice(0, 128), slice(128, 208))  # CL slices per chunk
NV_COLS = L + L * 2 * 3 + 3
S1 = 1.0 / 32.0            # h1 storage scale (fp8)
SR = 1.0 / 256.0           # r1 storage scale (fp8)
S2 = 1.0 / 64.0            # extra h2 eviction scale (net h2/2048)
GM = 1.0                   # gconv PSUM scale (h2 stays bf16)

_CACHED = None


def _build_nc():
    nc = bacc.Bacc("TRN2", target_bir_lowering=False)

    d = {}
    def din(name, shape, dt=F32R):
        d[name] = nc.dram_tensor(name, list(shape), dt, kind="ExternalInput")

    din("x0c0", (BPC, 128, N))
    din("x0c1", (BPC, 80, N))
    din("xa8c0", (BPC, 128, N), F8)
    din("xa8c1", (BPC, 80, N), F8)
    din("ph0", (DIMS, N + DIMS + BPC * DIMS))
    din("wfp", (128, 336))
    din("idenb", (128, 128), BF16)
    din("gcw0_0", (128, L * 128), BF16)
    din("gcw0_1", (80, L * 80), BF16)
    din("wg12_0", (128, L * 128), BF16)
    din("wg12_1", (80, L * 80), BF16)
    din("wh1a_0", (128, L * 2 * 128), F8)
    din("wh1a_1", (80, L * 2 * 80), F8)
    din("wfc2p_0", (128, 2 * 128), F8)
    din("wfc2p_1", (80, 2 * 80), F8)
    din("wskip_0", (128, L * 64), BF16)
    din("wskip_1", (80, L * 40), BF16)
    din("we1", (SKR, L * 64), BF16)
    din("we2t", (128, 12))
    din("vecs", (128, NV_COLS), F32)
    outp = nc.dram_tensor("outp", [BPC, 12, N], F32, kind="ExternalOutput")

    with TileContext(nc) as tc, \
         tc.tile_pool(name="wp", bufs=1) as wp, \
         tc.tile_pool(name="ap", bufs=1) as ap, \
         tc.tile_pool(name="pp", bufs=1, space="PSUM") as pp:

        def wtile(name, src_ap, shape, dt=F32R, eng=None):
            t = wp.tile(shape, dt, tag=name, name=name)
            (eng or nc.sync).dma_start(out=t[:], in_=src_ap)
            return t

        # phase0-critical loads packed into ONE DMA on the SP queue (each
        # DMA carries ~2.2us fixed latency); bulk weights on gpsimd.
        ph0 = wtile("ph0", d["ph0"][:], (DIMS, N + DIMS + BPC * DIMS),
                    eng=nc.sync)
        p2T = ph0[:, 0:N]
        p3sT = ph0[:, N:N + DIMS]
        adps = [ph0[:, N + DIMS + b * DIMS:N + DIMS + (b + 1) * DIMS]
                for b in range(BPC)]

        # trunk/attention tiles; x0 DMA'd straight into the first XT
        # buffers, with the att weights interleaved so att(0, b0) can start
        # before batch 1's x0 lands.
        xts = [[None, None] for _ in range(BPC)]
        for b in range(BPC):
            for c in range(2):
                rows = CH[c][1]
                xts[b][c] = ap.tile((rows, N), F32R, tag=f"XT{b}_{c}",
                                    bufs=2, name=f"XT{b}_{c}_init")
        nc.sync.dma_start(out=xts[0][0][:], in_=d["x0c0"][0])
        wfp = wtile("wfp", d["wfp"][:], (128, 336))
        wfc1 = [wfp[:, 0:128], wfp[:80, 128:208]]
        idenh = wfp[:, 208:336]
        nc.sync.dma_start(out=xts[0][1][:], in_=d["x0c1"][0])
        nc.sync.dma_start(out=xts[1][0][:], in_=d["x0c0"][1])
        nc.sync.dma_start(out=xts[1][1][:], in_=d["x0c1"][1])
        idenb = wtile("idenb", d["idenb"][:], (128, 128), BF16)
        vecs = wtile("vecs", d["vecs"][:], (128, NV_COLS), F32)

        php = [[ap.tile((CH[c][1], 2, N), F8, tag=f"PH{b}_{c}",
                        name=f"PH{b}_{c}") for c in range(2)]
               for b in range(BPC)]
        r1p = [[ap.tile((CH[c][1], 2, N), F8, tag=f"R1{b}_{c}",
                        name=f"R1{b}_{c}") for c in range(2)]
               for b in range(BPC)]

        h2t = [[ap.tile((CH[c][1], N), BF16, tag=f"H2{b}_{c}",
                        name=f"H2{b}_{c}") for c in range(2)]
               for b in range(BPC)]
        wfc2p = [wtile(f"wfc2p_{c}", d[f"wfc2p_{c}"][:],
                       (CH[c][1], 2, CH[c][1]), F8, eng=nc.gpsimd)
                 for c in range(2)]
        for b in range(BPC):
            for c in range(2):
                nc.gpsimd.dma_start(out=php[b][c][:, 1, :],
                                    in_=d[f"xa8c{c}"][b])
        gcw0 = [wtile(f"gcw0_{c}", d[f"gcw0_{c}"][:],
                      (CH[c][1], L, CH[c][1]), BF16, eng=nc.gpsimd)
                for c in range(2)]
        wg12 = [wtile(f"wg12_{c}", d[f"wg12_{c}"][:],
                      (CH[c][1], L, CH[c][1]), BF16, eng=nc.gpsimd)
                for c in range(2)]
        wh1a = [wtile(f"wh1a_{c}", d[f"wh1a_{c}"][:],
                      (CH[c][1], L, 2, CH[c][1]), F8, eng=nc.gpsimd)
                for c in range(2)]
        wskip = [wtile(f"wskip_{c}", d[f"wskip_{c}"][:],
                       (CH[c][1], L, (64, 40)[c]), BF16, eng=nc.gpsimd)
                 for c in range(2)]
        we1 = wtile("we1", d["we1"][:], (SKR, L, 64), BF16, eng=nc.gpsimd)
        we2t = wtile("we2t", d["we2t"][:], (128, 12), eng=nc.gpsimd)

        vc = {}
        ci = 0
        for i in range(L):
            vc[f"skb{i}"] = ci; ci += 1
        for i in range(L):
            for c in range(2):
                for nm in ("bns", "bnb", "bnsg"):
                    vc[f"{nm}{i}_{c}"] = ci; ci += 1
        vc["e1b"] = ci; ci += 1
        vc["e2b"] = ci; ci += 1
        vc["neg20"] = ci; ci += 1
        assert ci == NV_COLS

        def vcol(nm, rows=128):
            return vecs[:rows, vc[nm]:vc[nm] + 1]

        NS = (slice(0, 512), slice(512, 1024))
        BS = range(BPC)

        st = [dict() for _ in range(BPC)]
        # persistent fp8 adjacency in DoubleRow pair layout
        ATd = [[ap.tile((128, 2, N), F8, tag=f"ATd{b}_{k}", name=f"ATd{b}_{k}")
                for k in range(4)] for b in BS]
        ends = ap.tile((128, N), F32, tag="END", name="END")

        # PE p-state warmup: tiny matmuls as early as possible so the
        # 3us ramp window elapses before the heavy phase0 matmuls.
        def warmup():
            wps = pp.tile((DIMS, 8), F32, tag="pwork", bufs=4, name="warm")
            for r in range(6):
                nc.tensor.matmul(wps[:], adps[0][:], p2T[:, 0:8],
                                 start=(r == 0), stop=(r == 5))

        # ---------------- adjacency ----------------
        def phase0_pair():
            Lst = [ap.tile((64, N), F32R, tag=f"Lst{b}", name=f"Lst{b}")
                   for b in BS]
            Rst = [ap.tile((64, N), F32R, tag=f"Rst{b}", name=f"Rst{b}")
                   for b in BS]
            for nsi, ns in enumerate(NS):
                pss = []
                for b in BS:
                    ps = pp.tile((DIMS, 512), F32, tag="pwork", bufs=4,
                                 name=f"srcTps{b}_{nsi}")
                    nc.tensor.matmul(ps[:], adps[b], p2T[:, ns],
                                     start=True, stop=True)
                    pss.append(ps)
                for b in BS:
                    nc.scalar.activation(Rst[b][0:32, ns], pss[b][:], AF.Copy)
                    nc.vector.tensor_scalar(Lst[b][32:64, ns], pss[b][:],
                                            -1.0, None, ALU.mult)
            for nsi, ns in enumerate(NS):
                pss = []
                for b in BS:
                    ps = pp.tile((DIMS, 512), F32, tag="pwork", bufs=4,
                                 name=f"ups{b}_{nsi}")
                    nc.tensor.matmul(ps[:], p3sT, Rst[b][0:32, ns],
                                     start=True, stop=True)
                    pss.append(ps)
                for b in BS:
                    nc.scalar.activation(Lst[b][0:32, ns], pss[b][:], AF.Copy)
                    nc.vector.tensor_copy(Rst[b][32:64, ns], pss[b][:])
            st[0]["LR"] = (Lst, Rst)

        def phase0_D(fillers=()):
            fillers = list(fillers)
            Lst, Rst = st[0]["LR"]
            for v in range(8):
                cs = slice(v * 128, (v + 1) * 128)
                for nsi, ns in enumerate(NS):
                    dpss = []
                    for b in BS:
                        dps = pp.tile((128, 512), F32, tag="pwork", bufs=4,
                                      name=f"dps{b}_{v}_{nsi}")
                        nc.tensor.matmul(dps[:], Lst[b][:, cs], Rst[b][:, ns],
                                         start=True, stop=True)
                        dpss.append(dps)
                    # A = clamp(D, 0, 1): |D| ~ 5e4 >> 1 and off-diagonal
                    # |D| >= 0.03, so clamp == step(D>0) except on a ~2e-5
                    # sliver; diag D == 0 maps to 0 via the -20 bias. Act
                    # takes half the evictions as sigmoid(1e6 D - 20).
                    for b in BS:
                        if (b + v + nsi) % 2 == 0:
                            nc.vector.tensor_scalar(
                                ATd[b][v // 2][:, v % 2, ns], dpss[b][:],
                                0.0, 1.0, ALU.max, ALU.min)
                        else:
                            nc.scalar.activation(
                                ATd[b][v // 2][:, v % 2, ns], dpss[b][:],
                                AF.Sigmoid, bias=vcol("neg20"),
                                scale=1e6)
                if v % 2 == 1 and fillers:
                    fillers.pop(0)()
            while fillers:
                fillers.pop(0)()

        # ---------------- layer stages ----------------
        def att_c(i, c):
            """fc1 -> relu -> fc2 + x/2 -> sigmoid(2.) for one chunk."""
            rows = CH[c][1]
            xn = {}
            for b in BS:
                if c == 0:
                    st[b]["xn_next"] = [None, None]
                xn[b] = ap.tile((rows, N), BF16, tag=f"XN{b}_{c}",
                                bufs=2, name=f"XN{b}_{i}_{c}")
                st[b]["xn_next"][c] = xn[b]
            m1s, apss = {}, {}
            for nsi in range(2):
                ns = NS[nsi]
                for b in BS:
                    xt = st_xt(b)
                    m1 = pp.tile((rows, 512), F32, tag="pwork", bufs=4,
                                 name=f"m1_{b}_{i}_{c}_{nsi}")
                    nc.tensor.matmul(m1[:], wfc1[c], xt[c][:, ns],
                                     start=True, stop=True)
                    m1s[b, nsi] = m1
                for b in BS:
                    if b % 2 == 0:
                        nc.scalar.activation(r1p[b][c][:, 0, ns],
                                             m1s[b, nsi][:],
                                             AF.Relu, scale=SR)
                    else:
                        nc.vector.tensor_scalar(r1p[b][c][:, 0, ns],
                                                m1s[b, nsi][:],
                                                SR, 0.0,
                                                ALU.mult, ALU.max)
            for nsi in range(2):
                ns = NS[nsi]
                for b in BS:
                    xt = st_xt(b)
                    a_ps = pp.tile((rows, 512), F32, tag="pwork", bufs=4,
                                   name=f"aps{b}_{i}_{c}_{nsi}")
                    nc.tensor.matmul(a_ps[:], wfc2p[c][:, :, :],
                                     r1p[b][c][:, :, ns], perf_mode=DR,
                                     start=True, stop=False)
                    nc.tensor.matmul(a_ps[:], idenh[:rows, 0:rows],
                                     xt[c][:, ns], start=False, stop=True)
                    apss[b, nsi] = a_ps
                for b in BS:
                    nc.scalar.activation(xn[b][:, ns], apss[b, nsi][:],
                                         AF.Sigmoid, scale=2.0)

        def att(i):
            att_c(i, 0)
            att_c(i, 1)

        def st_xt(b):
            if "xt" not in st[b]:
                st[b]["xt"] = [xts[b][0], xts[b][1]]
            return st[b]["xt"]

        def tpx(i, b):
            """xn -> V-layout fp8 DoubleRow pairs xvd[kp] (128,2,CL)."""
            st[b]["xn"] = st[b]["xn_next"]
            xn = st[b]["xn"]
            xvd = [None] * 4
            for kp in range(4):
                tp = pp.tile((128, 2, CL), BF16, tag="ptr", bufs=2,
                             name=f"tpx{b}_{i}_{kp}")
                for s in range(2):
                    v = 2 * kp + s
                    cs = slice(v * 128, (v + 1) * 128)
                    for c in range(2):
                        o, rows = CH[c]
                        nc.tensor.transpose(tp[:, s, o:o + rows],
                                            xn[c][:, cs], idenb[:rows, :rows])
                xvd[kp] = ap.tile((128, 2, CL), F8, tag=f"XV{b}_{kp}",
                                  bufs=2, name=f"XV{b}_{i}_{kp}")
                if kp % 2 == 0:
                    nc.vector.tensor_copy(xvd[kp][:], tp[:])
                else:
                    nc.scalar.activation(xvd[kp][:], tp[:], AF.Copy)
            st[b]["xvd"] = xvd

        def hop1v_pb(i, p, b):
            """h1 V-pair for one w-pair p, one batch."""
            if p == 0:
                st[b]["h1d"] = [None] * 4
            xvd = st[b]["xvd"]
            h_ps = pp.tile((128, 2, CL), F32, tag="ptr", bufs=2,
                           name=f"hp{b}_{i}_{p}")
            for s in range(2):
                w = 2 * p + s
                ws = slice(w * 128, (w + 1) * 128)
                dst = h_ps[:, s, :]
                for kp in range(4):
                    nc.tensor.matmul(dst, ATd[b][kp][:, :, ws],
                                     xvd[kp][:], perf_mode=DR,
                                     start=(kp == 0), stop=(kp == 3))
            t = ap.tile((128, 2, CL), F8, tag=f"H1{b}_{p}",
                        bufs=2, name=f"H1{b}_{i}_{p}")
            if (p + b) % 2 == 0:
                nc.scalar.activation(t[:], h_ps[:], AF.Identity,
                                     scale=S1)
            else:
                nc.vector.tensor_scalar(t[:], h_ps[:], S1, None,
                                        ALU.mult)
            st[b]["h1d"][p] = t

        def hop1t_unit(i, c, nsi):
            rows, ns = CH[c][1], NS[nsi]
            g1s = {}
            for b in BS:
                xvd = st[b]["xvd"]
                g1 = pp.tile((rows, 512), F32, tag="pwork", bufs=4,
                             name=f"g1_{b}_{i}_{c}_{nsi}")
                for kp in range(4):
                    nc.tensor.matmul(g1[:], xvd[kp][:, :, CLS[c]],
                                     ATd[b][kp][:, :, ns], perf_mode=DR,
                                     start=(kp == 0), stop=(kp == 3))
                g1s[b] = g1
            for b in BS:
                dst = php[b][c][:, 0, ns]
                if (b + c + nsi) % 2 == 0:
                    nc.scalar.activation(dst, g1s[b][:], AF.Identity,
                                         scale=S1)
                else:
                    nc.vector.tensor_scalar(dst, g1s[b][:], S1, None,
                                            ALU.mult)

        def hop2t_unit(i, c, nsi):
            rows, ns = CH[c][1], NS[nsi]
            g2s = {}
            for b in BS:
                h1d = st[b]["h1d"]
                g2 = pp.tile((rows, 512), F32, tag="pwork", bufs=4,
                             name=f"g2_{b}_{i}_{c}_{nsi}")
                for kp in range(4):
                    nc.tensor.matmul(g2[:], h1d[kp][:, :, CLS[c]],
                                     ATd[b][kp][:, :, ns], perf_mode=DR,
                                     start=(kp == 0), stop=(kp == 3))
                g2s[b] = g2
            for b in BS:
                dst = h2t[b][c][:, ns]
                if (b + c + nsi) % 2 == 1:
                    nc.scalar.activation(dst, g2s[b][:], AF.Copy)
                else:
                    nc.vector.tensor_copy(dst, g2s[b][:])

        def skp_part(i):
            """skip conv -> relu (rsk)."""
            rsk = [ap.tile((SKR, N), BF16, tag=f"rsk{b}", bufs=2,
                           name=f"rsk{b}_{i}") for b in BS]
            sks = {}
            for nsi, ns in enumerate(NS):
                for b in BS:
                    xn = st[b]["xn"]
                    sk_ps = pp.tile((SKR, 512), F32, tag="pwork", bufs=4,
                                    name=f"skp{b}_{i}_{nsi}")
                    nc.tensor.matmul(sk_ps[:64], wskip[0][:, i, :],
                                     xn[0][:, ns], start=True, stop=True)
                    nc.tensor.matmul(sk_ps[64:], wskip[1][:, i, :],
                                     xn[1][:, ns], start=True, stop=True)
                    sks[b, nsi] = sk_ps
                for b in BS:
                    if b % 2 == 0:
                        nc.vector.tensor_scalar(rsk[b][:, ns], sks[b, nsi][:],
                                                vcol(f"skb{i}", SKR), 0.0,
                                                ALU.add, ALU.max)
                    else:
                        nc.scalar.activation(rsk[b][:, ns], sks[b, nsi][:],
                                             AF.Relu, bias=vcol(f"skb{i}", SKR))
            return rsk

        def eps_part(i, rsk):
            """end1 matmul on rsk; both batches stacked on the partition dim
            of one PSUM tile so a single DVE op accumulates them (cost is
            free-size-bound, partitions are parallel)."""
            for nsi, ns in enumerate(NS):
                e_ps = pp.tile((128, 512), F32, tag="pwork", bufs=4,
                               name=f"eps_{i}_{nsi}")
                for b in BS:
                    nc.tensor.matmul(e_ps[64 * b:64 * (b + 1), :],
                                     we1[:, i, :], rsk[b][:, ns],
                                     start=True, stop=True)
                if i == 0:
                    nc.vector.tensor_copy(ends[:, ns], e_ps[:])
                else:
                    nc.vector.scalar_tensor_tensor(
                        ends[:, ns], e_ps[:], 0.0,
                        ends[:, ns], ALU.bypass, ALU.add)

        def prenxs(i):
            for c in range(2):
                rows = CH[c][1]
                nxs = [ap.tile((rows, N), F32, tag=f"tmp{b}_{c}",
                               name=f"nxs{b}_{i}_{c}") for b in BS]
                for nsi, ns in enumerate(NS):
                    for b in BS:
                        xt = st_xt(b)
                        nc.gpsimd.tensor_scalar(
                            nxs[b][:, ns], xt[c][:, ns].bitcast(F32),
                            vcol(f"bns{i}_{c}", rows),
                            vcol(f"bnb{i}_{c}", rows), ALU.mult, ALU.add)
                for b in BS:
                    st[b].setdefault("nxs", [None, None])[c] = nxs[b]

        def gconv_unit(i, c, nsi):
            rows, ns = CH[c][1], NS[nsi]
            if nsi == 0:
                for b in BS:
                    st[b].setdefault("nxt", [None, None])[c] = ap.tile(
                        (rows, N), F32R, tag=f"XT{b}_{c}", bufs=2,
                        name=f"XT{b}_{i}_{c}")
            gps = []
            for b in BS:
                xn = st[b]["xn"]
                g_ps = pp.tile((rows, 512), F32, tag="pg", bufs=2,
                               name=f"gp{b}_{i}_{c}_{nsi}")
                nc.tensor.matmul(g_ps[:], gcw0[c][:, i, :],
                                 xn[c][:, ns], start=True, stop=False)
                nc.tensor.matmul(g_ps[:], wh1a[c][:, i, :, :],
                                 php[b][c][:, :, ns], perf_mode=DR,
                                 start=False, stop=False)
                nc.tensor.matmul(g_ps[:], wg12[c][:, i, :],
                                 h2t[b][c][:, ns],
                                 start=False, stop=True)
                gps.append(g_ps)
            for b in BS:
                nc.vector.scalar_tensor_tensor(
                    st[b]["nxt"][c][:, ns], gps[b][:],
                    vcol(f"bnsg{i}_{c}", rows), st[b]["nxs"][c][:, ns],
                    ALU.mult, ALU.add)
            if nsi == 1:
                for b in BS:
                    st_xt(b)[c] = st[b]["nxt"][c]

        # ---------------- end convs        # ---------------- end convs ----------------
        def tail():
            o1m = ap.tile((128, N), F32R, tag="o1", name="o1m")
            obs = {b: ap.tile((12, N), F32, tag=f"ob{b}", name=f"ob{b}")
                   for b in BS}
            # ends is batch-stacked and e1b/we2t are already duplicated
            # across both partition halves: one relu covers both batches.
            for nsi, ns in enumerate(NS):
                nc.scalar.activation(o1m[:, ns], ends[:, ns],
                                     AF.Relu, bias=vcol("e1b", 128))
            for nsi, ns in enumerate(NS):
                for b in BS:
                    o2_ps = pp.tile((12, 512), F32, tag="pwork", bufs=4,
                                    name=f"o2p{b}_{nsi}")
                    nc.tensor.matmul(o2_ps[:],
                                     we2t[64 * b:64 * (b + 1), :],
                                     o1m[64 * b:64 * (b + 1), ns],
                                     start=True, stop=True)
                    nc.vector.tensor_scalar(obs[b][:, ns], o2_ps[:],
                                            vcol("e2b", 12), None,
                                            ALU.add)
                    nc.sync.dma_start(out=outp[b][:, ns], in_=obs[b][:, ns])

        # ---------------- emission ----------------
        # Dummy sigmoid as the very first Act op: pins the
        # sigmoid_and_others activation table (which covers every function
        # this kernel uses) so only one table load is ever issued, and it
        # happens during the initial DMA wait.
        actwarm = ap.tile((1, 8), F32, tag="actwarm", name="actwarm")
        nc.vector.memset(actwarm[:], 0.0)
        nc.scalar.activation(actwarm[:], actwarm[:], AF.Sigmoid)
        phase0_pair()
        for b in BS:
            for c in range(2):
                # plane 1 only needs finite contents (stationary is zero);
                # deferred past phase0_pair so its DVE evictions go first
                nc.vector.memset(r1p[b][c][:, 1, :], 0.0)
        att(0)
        phase0_D(fillers=[lambda: tpx(0, 0), lambda: tpx(0, 1)])
        for i in range(L):
            if i == L - 1:
                # the trunk/hop outputs of the last layer are dead code:
                # only its attention + skip contribution reach the output
                for b in BS:
                    st[b]["xn"] = st[b]["xn_next"]
                eps_part(i, skp_part(i))
                break
            if i > 0:
                for b in BS:
                    tpx(i, b)
            prenxs(i)
            for p, b in ((0, 0), (1, 0), (0, 1), (2, 0),
                         (1, 1), (3, 0), (2, 1), (3, 1)):
                hop1v_pb(i, p, b)
            rsk = skp_part(i)
            eps_part(i, rsk)
            for c in range(2):
                for nsi in range(2):
                    hop1t_unit(i, c, nsi)
            hop2t_unit(i, 0, 0)
            hop2t_unit(i, 0, 1)
            gconv_unit(i, 0, 0)
            gconv_unit(i, 0, 1)
            hop2t_unit(i, 1, 0)
            hop2t_unit(i, 1, 1)
            att_c(i + 1, 0)
            gconv_unit(i, 1, 0)
            gconv_unit(i, 1, 1)
            att_c(i + 1, 1)
        tail()

    nc.finalize()
    return nc


# ----------------------------------------------------------------------------
# host-side preprocessing
# ----------------------------------------------------------------------------

def _prep_host(inputs):
    f = lambda x: np.asarray(x, dtype=np.float32)
    bf = lambda x: np.ascontiguousarray(x).astype(ml_dtypes.bfloat16)
    f8 = lambda x: np.ascontiguousarray(x).astype(ml_dtypes.float8_e4m3)
    f85 = lambda x: np.ascontiguousarray(x).astype(ml_dtypes.float8_e5m2)
    x_in = f(inputs["inputs"])
    ind = np.asarray(inputs["ind"]).astype(np.int64)
    p1, p2, p3, pk = f(inputs["p1"]), f(inputs["p2"]), f(inputs["p3"]), f(inputs["pk"])

    xo = np.pad(x_in, ((0, 0), (0, 0), (0, 0), (RF - T, 0)))
    xo_t = xo.transpose(0, 1, 3, 2)               # (B, 2, RF, N)
    te = p1[ind]
    adp = np.einsum("bi,ijk->bjk", te, pk).astype(np.float32)

    start_w, start_b = f(inputs["start_w"]), f(inputs["start_b"])
    starta_w, starta_b = f(inputs["starta_w"]), f(inputs["starta_b"])
    fc1_w, fc2_w = f(inputs["fc1_w"]), f(inputs["fc2_w"])
    skip_w, skip_b = f(inputs["skip_w"]), f(inputs["skip_b"])
    gconv_w, gconv_b = f(inputs["gconv_w"]), f(inputs["gconv_b"])
    bn_g, bn_b = f(inputs["bn_g"]), f(inputs["bn_b"])
    bna_g, bna_b = f(inputs["bna_g"]), f(inputs["bna_b"])
    end1_w, end1_b = f(inputs["end1_w"]), f(inputs["end1_b"])
    end2_w, end2_b = f(inputs["end2_w"]), f(inputs["end2_b"])

    # start convs on host: l-major T-layout rows (l*16+ch)
    x0 = (start_w[:, 0][None, None, :, None] * xo_t[:, 0][:, :, None, :]
          + start_b[None, None, :, None]).reshape(B, CL, N)
    xa = (starta_w[:, 0][None, None, :, None] * xo_t[:, 1][:, :, None, :]
          + starta_b[None, None, :, None]).reshape(B, CL, N)
    xa8 = f8(xa)

    e8, e5 = np.eye(8, dtype=np.float32), np.eye(5, dtype=np.float32)
    kr = lambda e, w: np.kron(e, np.ascontiguousarray(w.T)).astype(np.float32)

    bns = (bn_g / np.sqrt(1.0 + BN_EPS)).astype(np.float32)
    bnas = (bna_g / np.sqrt(1.0 + BN_EPS)).astype(np.float32)

    # per-layer xa scale av and folded bias bv
    avs, bvs = [np.ones(16, dtype=np.float32)], [np.zeros(16, dtype=np.float32)]
    for i in range(L):
        avs.append(2.0 * bnas[i] * avs[i])
        bvs.append(2.0 * bnas[i] * bvs[i] + bna_b[i])

    gcw0_c, wg12_c, wh1a_c, wskip_c, wfc2p_c = [], [], [], [], []
    for c, (e, rows, reps) in enumerate(((e8, 128, 8), (e5, 80, 5))):
        f2 = np.stack([kr(e, fc2_w) / SR,
                       np.zeros((rows, rows), dtype=np.float32)], axis=1)
        wfc2p_c.append(f8(f2.reshape(rows, 2 * rows)))
        g0 = np.stack([kr(e, gconv_w[i][:, 0:16]) for i in range(L)],
                      axis=1)
        g1 = np.stack([kr(e, gconv_w[i][:, 16:32]) / S1
                      for i in range(L)], axis=1)
        g2 = np.stack([kr(e, gconv_w[i][:, 32:48]) / S1
                       for i in range(L)], axis=1)
        wavm = np.stack([np.diag(np.tile(avs[i], reps))
                         for i in range(L)], axis=1)   # (rows, L, rows)
        wh = np.stack([g1, wavm], axis=2)              # (rows, L, 2, rows)
        wsk = np.stack([kr(e, skip_w[i]) for i in range(L)], axis=1)
        gcw0_c.append(bf(g0.reshape(rows, L * rows)))
        wg12_c.append(bf(g2.reshape(rows, L * rows)))
        wh1a_c.append(f8(wh.reshape(rows, L * 2 * rows)))
        wskip_c.append(bf(wsk.reshape(rows, L * (64, 40)[c])))

    # end1 columns: ref skip rows are o*13+l within the (L-1-i)-th block;
    # ours are l*8+o
    we1 = np.zeros((SKR, L, 64), dtype=np.float32)
    ll, oo = np.meshgrid(np.arange(RF), np.arange(SC), indexing="ij")
    src_col = oo.ravel() * RF + ll.ravel()
    for i in range(L):
        we1[:, i, :] = end1_w[:, (L - 1 - i) * SKR + src_col].T

    t8 = lambda v: np.tile(v, 8)
    vecs = np.zeros((128, NV_COLS), dtype=np.float32)
    ci = 0
    for i in range(L):
        vecs[:SKR, ci] = np.tile(skip_b[i], RF); ci += 1
    for i in range(L):
        bnb_adj = bn_b[i] + bns[i] * (gconv_b[i] + bvs[i])
        vecs[:, ci] = t8(bns[i]); ci += 1
        vecs[:, ci] = t8(bnb_adj); ci += 1
        vecs[:, ci] = t8(GM * bns[i]); ci += 1
        vecs[:80, ci] = np.tile(bns[i], 5); ci += 1
        vecs[:80, ci] = np.tile(bnb_adj, 5); ci += 1
        vecs[:80, ci] = np.tile(GM * bns[i], 5); ci += 1
    vecs[:64, ci] = end1_b
    vecs[64:128, ci] = end1_b; ci += 1
    vecs[:12, ci] = end2_b; ci += 1
    vecs[:, ci] = -20.0; ci += 1
    assert ci == NV_COLS

    shared = {
        "wfp": np.concatenate(
            [kr(e8, fc1_w),
             np.pad(kr(e5, fc1_w), ((0, 48), (0, 0))),
             0.5 * np.eye(128, dtype=np.float32)], axis=1),
        "idenb": np.eye(128, dtype=ml_dtypes.bfloat16),
        "gcw0_0": gcw0_c[0], "gcw0_1": gcw0_c[1],
        "wg12_0": wg12_c[0], "wg12_1": wg12_c[1],
        "wh1a_0": wh1a_c[0], "wh1a_1": wh1a_c[1],
        "wskip_0": wskip_c[0], "wskip_1": wskip_c[1],
        "wfc2p_0": wfc2p_c[0], "wfc2p_1": wfc2p_c[1],
        "we1": bf(we1.reshape(SKR, L * 64)),
        "we2t": np.concatenate([end2_w.T, end2_w.T], axis=0).astype(np.float32),
        "vecs": vecs,
    }
    in_maps = []
    for cix in range(NCORES):
        bs = slice(cix * BPC, (cix + 1) * BPC)
        m = dict(shared)
        m["x0c0"] = np.ascontiguousarray(x0[bs, 0:128])
        m["x0c1"] = np.ascontiguousarray(x0[bs, 128:208])
        m["xa8c0"] = np.ascontiguousarray(xa8[bs, 0:128])
        m["xa8c1"] = np.ascontiguousarray(xa8[bs, 128:208])
        m["ph0"] = np.ascontiguousarray(np.concatenate(
            [p2.T, p3[:DIMS, :DIMS].T,
             adp[bs].transpose(1, 0, 2).reshape(DIMS, BPC * DIMS)], axis=1))
        in_maps.append(m)
    return in_maps


def _get_nc():
    global _CACHED
    if _CACHED is None:
        _CACHED = _build_nc()
    return _CACHED


def run(inputs, trace=False):
    nc = _get_nc()
    in_maps = _prep_host(inputs)
    res = run_bass_kernel_spmd(nc, in_maps, core_ids=list(range(NCORES)),
                               trace=trace)
    out = np.stack([res.results[c]["outp"] for c in range(NCORES)])
    out = out.reshape(B, 12, N, 1).astype(np.float32)
    return out, res


def kernel(**inputs):
    out, _ = run(inputs)
    return out

